# revision 63
# baseline (speedup 1.0000x reference)
"""Trainium2 Bass kernel for nn_AttentionBlock_9792525435528.

Reference computation (per batch element b):
    xf = x[b].reshape(C, T)                      # C=512, T=32*32=1024
    GroupNorm(G=32) -> xn
    qkv = qkv_w @ xn + qkv_b                     # [3C, T]
    per head h (NH=8, ch=64): q,k,v; w = softmax((q*s)^T (k*s)); a = v @ w^T
    h = proj_w @ a + proj_b
    out = (xf + h) / sqrt(2)

Sharding: data-parallel over batch. 8 batch elements -> 8 NeuronCores, one
each. Weights replicated. No cross-core communication needed.

Device algorithm (fp8 + pipelined):
  - QKV and attention-MM2 run in fp8e4 with DoubleRow perf mode (2 rows of
    the contraction per PE cell -> half the matmul instructions). q/k stay
    bf16 for MM1 (64-row contraction cannot pack); proj stays bf16 for
    accuracy (it feeds the residual directly).
  - GroupNorm statistics stream behind the x DMA chunk-by-chunk; rstd uses
    the 0x5f3759df magic-constant rsqrt + a Newton step entirely on
    VectorE, so no ScalarE table set other than exp is ever loaded and the
    critical path never hops engines.
  - x streams in as eight half-chunk DMAs round-robined over the three
    DMA-capable queues (Sync/ScalarE/GpSimd); constants + weights issue on
    GpSimd behind the vT memset so they don't steal HBM bandwidth from x.
  - A burst of (nonzero-data) dummy matmuls at t=0 burns the HAM clock
    gate's cold window; v and the later qk projections are spread through
    the MM1 stretches of pairs 0-1 as full-activity filler so the
    activity monitor keeps the PE at 2.4 GHz.
  - Attention softmax: w^T[s,t] layout; exp tiles are split between ScalarE
    (table exp -> fp8 out) and VectorE (Schraudolph bit-trick exp: one
    tensor_scalar writing int8 bits of fp8e4). The softmax denominator
    comes for free from 64 lambda-columns appended to v^T in MM2 (the fp8
    v-scale lambda cancels in the num/den ratio).
  - MM2 epilogue: both half-pair denominators stack into one [128,512]
    tile, one reciprocal_approx_fast per t-chunk, two normalizing muls.
  - Weight scale lambda=16 on wq/wk/wv keeps fp8 weights out of the
    subnormal range; the q*k logit scale (QK_SCALE^2 * lambda^-2 = 1/2048)
    is folded into the exp's affine pre-scale.
"""

import ml_dtypes
import numpy as np

import concourse.bass as bass
import concourse.mybir as mybir
import concourse.tile as tile
from concourse import bacc
from concourse.bass_utils import run_bass_kernel_spmd

B, C, T = 8, 512, 1024
NH, CH, G = 8, 64, 32
GS = C // G  # 16 channels per group
EPS = 1e-6
NCORES = 8
P = 128
KC = C // P   # 4 chunks of 128 channels
SCN = T // P  # 8 s-chunks
SCP = SCN // 2  # 4 s-chunk pairs (DoubleRow)
NT = T // 512   # 2 t-chunks of 512
ISQ2 = float(1.0 / np.sqrt(2.0))
QK_SCALE2 = float(1.0 / np.sqrt(CH))  # (1/sqrt(sqrt(ch)))^2
LAM = 16.0  # fp8 scale for wq/wk/wv (keeps weights out of subnormals)
EXP_SCALE = QK_SCALE2 / (LAM * LAM)   # = 1/2048
LOG2E = 1.4426950408889634
SCH_C = 0.05  # Schraudolph bias correction (device cast = round-to-nearest)
RSQRT_MAGIC = 0x5F3759DF

F32 = mybir.dt.float32
BF16 = mybir.dt.bfloat16
FP8 = mybir.dt.float8e4
I8 = mybir.dt.int8
I16 = mybir.dt.int16
I32 = mybir.dt.int32

N_DUMMY = 16    # HAM warm-up matmuls at t=0
N_DUMMY_TAIL = 0   # tail runs throttled regardless; dummies there cost 2x
# exp tiles handed to VectorE (Schraudolph) per pair; rest go to ScalarE.
# The last pair gets more DVE so the tail drains on both engines.
EXPS_DVE = {
    0: {(1, 1), (2, 1), (3, 1), (5, 1), (6, 1), (7, 1)},
    1: {(1, 1), (3, 1), (4, 1), (5, 1), (7, 1)},
    2: {(1, 1), (3, 1), (4, 1), (5, 1), (7, 1)},
    3: {(0, 1), (1, 1), (3, 1), (4, 1), (5, 1), (7, 1)},
}

_GRAPH_CACHE = {}


def _build_graph(qkv_bias_nz: bool, proj_bias_nz: bool, debug_taps: bool = False):
    nc = bacc.Bacc("TRN2", target_bir_lowering=False, debug=False)
    AF = mybir.ActivationFunctionType
    ALU = mybir.AluOpType
    DR = mybir.MatmulPerfMode.DoubleRow

    # ---- DRAM I/O ------------------------------------------------------
    x_d = nc.dram_tensor("x", [C, T], F32, kind="ExternalInput").ap()
    wq_d = nc.dram_tensor("wqT", [C, C], FP8, kind="ExternalInput").ap()
    wk_d = nc.dram_tensor("wkT", [C, C], FP8, kind="ExternalInput").ap()
    wv_d = nc.dram_tensor("wvT", [C, C], FP8, kind="ExternalInput").ap()
    pw_d = nc.dram_tensor("pwT", [C, C], BF16, kind="ExternalInput").ap()
    gnw_d = nc.dram_tensor("gnw", [C], F32, kind="ExternalInput").ap()
    gnb_d = nc.dram_tensor("gnb", [C], F32, kind="ExternalInput").ap()
    ind16_d = nc.dram_tensor("ind16", [C, G], F32, kind="ExternalInput").ap()
    indT_d = nc.dram_tensor("indT", [G, C], F32, kind="ExternalInput").ap()
    qb_d = kb_d = vb_d = pb_d = None
    if qkv_bias_nz:
        qb_d = nc.dram_tensor("qb", [C], F32, kind="ExternalInput").ap()
        kb_d = nc.dram_tensor("kb", [C], F32, kind="ExternalInput").ap()
        vb_d = nc.dram_tensor("vb", [C], F32, kind="ExternalInput").ap()
    if proj_bias_nz:
        pb_d = nc.dram_tensor("pb", [C], F32, kind="ExternalInput").ap()
    out_d = nc.dram_tensor("out", [C, T], F32, kind="ExternalOutput").ap()
    dbg = {}
    if debug_taps:
        dbg["xn"] = nc.dram_tensor("dbg_xn", [P, KC, T], mybir.dt.uint8, kind="ExternalOutput").ap()
        dbg["q"] = nc.dram_tensor("dbg_q", [P, KC, T], BF16, kind="ExternalOutput").ap()
        dbg["k"] = nc.dram_tensor("dbg_k", [P, KC, T], BF16, kind="ExternalOutput").ap()
        dbg["vt"] = nc.dram_tensor("dbg_vt", [P, SCN, NH * P], mybir.dt.uint8, kind="ExternalOutput").ap()
        dbg["ew0"] = nc.dram_tensor("dbg_ew0", [SCN, P, T], mybir.dt.uint8, kind="ExternalOutput").ap()
        dbg["a"] = nc.dram_tensor("dbg_a", [P, KC, T], BF16, kind="ExternalOutput").ap()

    with tile.TileContext(nc) as tc:
        with (
            tc.tile_pool(name="big", bufs=1) as big,
            tc.tile_pool(name="wpool", bufs=1) as wpool,
            tc.tile_pool(name="small", bufs=1) as small,
            tc.tile_pool(name="ew", bufs=16) as ewpool,
            tc.tile_pool(name="dn", bufs=2) as dnpool,
            tc.tile_pool(name="ps1", bufs=2, space="PSUM") as ps1,
            tc.tile_pool(name="ps2", bufs=3, space="PSUM") as ps2,
            tc.tile_pool(name="psd", bufs=1, space="PSUM") as psd,
        ):
            # ---- phase 0: warmups + loads -----------------------------
            # x is the GroupNorm critical path: eight half-chunk DMAs
            # round-robin across the three DMA-capable queues (Sync,
            # ScalarE, GpSimd) so both descriptor generation and the
            # per-queue transfer bandwidth parallelize.
            x_sb = big.tile([P, KC, T], F32, tag="x")
            x_dr = x_d.rearrange("(o p) t -> p o t", p=P)
            dma_engs = (nc.sync, nc.scalar, nc.gpsimd)
            for i, (o, hlf) in enumerate((o, h) for o in range(KC) for h in range(2)):
                dma_engs[i % 3].dma_start(
                    out=x_sb[:, o, hlf * 512:(hlf + 1) * 512],
                    in_=x_dr[:, o, hlf * 512:(hlf + 1) * 512],
                )

            # Dummy matmuls burn the HAM cold window while DMAs land. All
            # dummies write one dedicated PSUM bank so they never couple
            # to the real tile rings (a dummy must never wait on another
            # engine -- its whole job is keeping the PE clock gate open).
            # non-zero, per-lane-varying data: the activity monitor the
            # clock gate feeds on appears to weight actual datapath
            # toggling, so all-zero dummies would read as idle
            dmy_sb = small.tile([P, 640], BF16, tag="dmy")
            nc.gpsimd.iota(
                dmy_sb, pattern=[[1, 640]], base=1, channel_multiplier=3,
                allow_small_or_imprecise_dtypes=True,
            )
            pdmy = psd.tile([P, 512], F32, tag="dmy")

            def emit_dummies(n):
                for _ in range(n):
                    nc.tensor.matmul(
                        pdmy, lhsT=dmy_sb[:, 0:128], rhs=dmy_sb[:, 128:640],
                        start=True, stop=True,
                    )

            emit_dummies(N_DUMMY)

            # Warm the exp table set (the only ACT_TABLE_LOAD in the kernel).
            warm = small.tile([G, 1], F32, tag="warm")
            nc.vector.memset(warm, 1.0)
            nc.scalar.activation(out=warm, in_=warm, func=AF.Exp)

            # GpSimd issues the tiny constants, then stalls on the big vT
            # memset before issuing the weight DMAs -- which keeps the
            # weight transfers from stealing HBM bandwidth from x.
            gnw_sb = small.tile([P, KC], F32, tag="gnw")
            nc.gpsimd.dma_start(out=gnw_sb, in_=gnw_d.rearrange("(o p) -> p o", p=P))
            gnb_sb = small.tile([P, KC], F32, tag="gnb")
            nc.gpsimd.dma_start(out=gnb_sb, in_=gnb_d.rearrange("(o p) -> p o", p=P))
            ind16_sb = small.tile([P, KC, G], F32, tag="ind16")
            nc.gpsimd.dma_start(
                out=ind16_sb, in_=ind16_d.rearrange("(o p) g -> p o g", p=P)
            )
            indT_sb = small.tile([G, KC, P], F32, tag="indT")
            nc.gpsimd.dma_start(out=indT_sb, in_=indT_d.rearrange("g (o p) -> g o p", p=P))

            # v^T augmented: per head 128 cols = [64 v^T cols | 64 lambda].
            vT_sb = big.tile([P, SCN, NH * P], FP8, tag="vT")
            vT4 = vT_sb.rearrange("p s (h z) -> p s h z", z=P)
            nc.gpsimd.memset(vT4[:, :, :, CH:P], LAM)

            wq_sb = wpool.tile([P, KC, C], FP8, tag="wq")
            nc.gpsimd.dma_start(out=wq_sb, in_=wq_d.rearrange("(o p) n -> p o n", p=P))
            wk_sb = wpool.tile([P, KC, C], FP8, tag="wk")
            nc.gpsimd.dma_start(out=wk_sb, in_=wk_d.rearrange("(o p) n -> p o n", p=P))
            wv_sb = wpool.tile([P, KC, C], FP8, tag="wv")
            nc.gpsimd.dma_start(out=wv_sb, in_=wv_d.rearrange("(o p) n -> p o n", p=P))
            pw_sb = wpool.tile([P, KC, C], BF16, tag="pw")
            nc.gpsimd.dma_start(out=pw_sb, in_=pw_d.rearrange("(o p) n -> p o n", p=P))

            bias_aps = {}
            for nm, d in (("qb", qb_d), ("kb", kb_d), ("pb", pb_d)):
                if d is not None:
                    t_ = small.tile([P, KC], F32, tag=nm)
                    nc.gpsimd.dma_start(out=t_, in_=d.rearrange("(o p) -> p o", p=P))
                    bias_aps[nm] = t_
            if vb_d is not None:
                vb_bc = small.tile([P, C], F32, tag="vb")
                nc.gpsimd.dma_start(
                    out=vb_bc,
                    in_=bass.AP(tensor=vb_d.tensor, offset=vb_d.offset,
                                ap=[[0, P]] + vb_d.ap),
                )
                bias_aps["vb"] = vb_bc

            # ---- phase 1: GroupNorm (streams behind the x DMA) --------
            # GroupNorm groups (16 consecutive channels) never cross the
            # 128-channel chunks, so the whole normalization splits into
            # two independent half-pipelines over chunk pairs (0,1) and
            # (2,3): each half finishes its stats -> rsqrt -> broadcast ->
            # xn as soon as its own chunks land, instead of waiting for
            # all of x.
            stats6 = small.tile([P, KC, 2, 6], F32, tag="stats6")
            stats2 = small.tile([P, KC, 2], F32, tag="stats2")
            mtmp = small.tile([P, KC], F32, tag="mtmp")
            A_sb = small.tile([P, KC], F32, tag="A")
            B_sb = small.tile([P, KC], F32, tag="B")
            xn_sb = big.tile([P, KC, T], FP8, tag="xn")
            GH = G // 2  # 16 groups per half
            for half in range(2):
                for o in (2 * half, 2 * half + 1):
                    for hlf in range(2):
                        nc.vector.bn_stats(
                            out=stats6[:, o, hlf, :],
                            in_=x_sb[:, o, hlf * 512:(hlf + 1) * 512],
                        )
                    # stats2 = (mean, E[x^2]): aggr writes (mean, var) in
                    # place, then var += mean^2
                    nc.vector.bn_aggr(out=stats2[:, o, :], in_=stats6[:, o, :, :])
                    nc.vector.tensor_mul(
                        out=mtmp[:, o:o + 1], in0=stats2[:, o, 0:1],
                        in1=stats2[:, o, 0:1],
                    )
                    nc.vector.tensor_add(
                        out=stats2[:, o, 1:2], in0=stats2[:, o, 1:2],
                        in1=mtmp[:, o:o + 1],
                    )
                psum_s = ps2.tile([GH, 2], F32, tag="ps2", name=f"psum_s{half}")
                for o in (2 * half, 2 * half + 1):
                    nc.tensor.matmul(
                        psum_s,
                        lhsT=ind16_sb[:, o, half * GH:(half + 1) * GH],
                        rhs=stats2[:, o, :],
                        start=(o == 2 * half),
                        stop=(o == 2 * half + 1),
                    )

                musd = small.tile([GH, 2], F32, tag=f"musd{half}", name=f"musd{half}")
                nc.vector.tensor_copy(out=musd, in_=psum_s)  # (mu, E[x^2])
                vg = small.tile([GH, 5], F32, tag=f"vg{half}", name=f"vg{half}")
                x_c, y_c, a_c, b_c, c_c = (vg[:, i:i + 1] for i in range(5))
                nc.vector.tensor_mul(out=x_c, in0=musd[:, 0:1], in1=musd[:, 0:1])
                nc.vector.tensor_sub(out=x_c, in0=musd[:, 1:2], in1=x_c)
                nc.vector.tensor_scalar(
                    out=x_c, in0=x_c, scalar1=EPS, scalar2=None, op0=ALU.add
                )  # x = var + eps
                nc.vector.tensor_scalar(
                    out=y_c.bitcast(I32), in0=x_c.bitcast(I32),
                    scalar1=1, scalar2=None, op0=ALU.logical_shift_right,
                )
                nc.vector.tensor_scalar(
                    out=y_c.bitcast(I32), in0=y_c.bitcast(I32),
                    scalar1=-1, scalar2=RSQRT_MAGIC,
                    op0=ALU.mult, op1=ALU.add,
                )  # y0 = magic - (bits >> 1)
                # one Newton step: rstd accurate to ~0.2% (xn-path only)
                nc.vector.tensor_mul(out=a_c, in0=x_c, in1=y_c)
                nc.vector.tensor_mul(out=b_c, in0=a_c, in1=y_c)
                nc.vector.tensor_scalar(
                    out=c_c, in0=b_c, scalar1=-0.5, scalar2=1.5,
                    op0=ALU.mult, op1=ALU.add,
                )
                nc.vector.tensor_mul(out=musd[:, 1:2], in0=y_c, in1=c_c)

                # broadcast (mu, rstd) to this half's channels: indT rows
                # for half-1 groups are copied into partitions 0:16 of
                # indT_sb at load (see indT host layout), so lhsT/rhs base
                # partitions always match.
                psum_b = ps2.tile([P, 4], F32, tag="ps2", name=f"psum_b{half}")
                for oi, o in enumerate((2 * half, 2 * half + 1)):
                    nc.tensor.matmul(
                        psum_b[:, oi * 2:(oi + 1) * 2],
                        lhsT=indT_sb[0:GH, 2 * half + oi, :], rhs=musd,
                        start=True, stop=True,
                    )
                musd_c = small.tile([P, 2, 2], F32, tag=f"musd_c{half}",
                                    name=f"musd_c{half}")
                nc.vector.tensor_copy(
                    out=musd_c, in_=psum_b.rearrange("p (o c) -> p o c", c=2)
                )

                # A = rstd * gn_w ; B = gn_b - mu * A for this half
                osl = slice(2 * half, 2 * half + 2)
                nc.vector.tensor_mul(out=A_sb[:, osl], in0=musd_c[:, :, 1],
                                     in1=gnw_sb[:, osl])
                nc.vector.tensor_mul(out=B_sb[:, osl], in0=musd_c[:, :, 0],
                                     in1=A_sb[:, osl])
                nc.vector.tensor_sub(out=B_sb[:, osl], in0=gnb_sb[:, osl],
                                     in1=B_sb[:, osl])

                # xn = x*A + B -> fp8; first chunk of the half on ScalarE,
                # second on VectorE
                for oi, o in enumerate((2 * half, 2 * half + 1)):
                    if oi == 0:
                        nc.scalar.activation(
                            out=xn_sb[:, o, :], in_=x_sb[:, o, :],
                            func=AF.Identity,
                            bias=B_sb[:, o:o + 1], scale=A_sb[:, o:o + 1],
                        )
                    else:
                        nc.vector.tensor_scalar(
                            out=xn_sb[:, o, :], in0=x_sb[:, o, :],
                            scalar1=A_sb[:, o:o + 1], scalar2=B_sb[:, o:o + 1],
                            op0=ALU.mult, op1=ALU.add,
                        )

            # ---- phase 2: QKV projections (fp8 DoubleRow) -------------
            q_sb = big.tile([P, KC, T], BF16, tag="q")
            k_sb = big.tile([P, KC, T], BF16, tag="k")

            def emit_qk_tile(j, which, t):
                """One q-or-k output tile of head pair j: 2 DR matmuls + a
                PSUM->bf16 cast. ~0.5us of full-activity PE work."""
                dst, w_sb, bias = (
                    (q_sb, wq_sb, bias_aps.get("qb")) if which == "q"
                    else (k_sb, wk_sb, bias_aps.get("kb"))
                )
                pq = ps2.tile([P, 512], F32, tag="ps2")
                for kk in range(KC // 2):
                    nc.tensor.matmul(
                        pq,
                        lhsT=w_sb[:, 2 * kk:2 * kk + 2, j * P:(j + 1) * P],
                        rhs=xn_sb[:, 2 * kk:2 * kk + 2, t * 512:(t + 1) * 512],
                        start=(kk == 0),
                        stop=(kk == KC // 2 - 1),
                        perf_mode=DR,
                    )
                dslice = dst[:, j, t * 512:(t + 1) * 512]
                if bias is not None:
                    nc.vector.tensor_scalar(
                        out=dslice, in0=pq, scalar1=bias[:, j:j + 1],
                        scalar2=None, op0=ALU.add,
                    )
                else:
                    nc.vector.tensor_copy(out=dslice, in_=pq)

            def emit_qk(j):
                for which in ("q", "k"):
                    for t in range(NT):
                        emit_qk_tile(j, which, t)

            def emit_v_tile(sc):
                pv = ps2.tile([P, 512], F32, tag="ps2")
                for kk in range(KC // 2):
                    nc.tensor.matmul(
                        pv,
                        lhsT=xn_sb[:, 2 * kk:2 * kk + 2, sc * P:(sc + 1) * P],
                        rhs=wv_sb[:, 2 * kk:2 * kk + 2, :],
                        start=(kk == 0),
                        stop=(kk == KC // 2 - 1),
                        perf_mode=DR,
                    )
                vdst = vT4[:, sc, :, 0:CH]  # [P, NH, CH] strided dst
                if "vb" in bias_aps:
                    nc.vector.scalar_tensor_tensor(
                        out=vdst, in0=pv.rearrange("p (h z) -> p h z", z=CH),
                        scalar=0.0,
                        in1=bias_aps["vb"].rearrange("p (h z) -> p h z", z=CH),
                        op0=ALU.add, op1=ALU.add,
                    )
                elif sc % 2 == 0:
                    # v runs as pair-0 filler: split the PSUM->fp8 copies
                    # across both elementwise engines
                    nc.scalar.copy(
                        out=vdst, in_=pv.rearrange("p (h z) -> p h z", z=CH)
                    )
                else:
                    nc.vector.tensor_copy(
                        out=vdst, in_=pv.rearrange("p (h z) -> p h z", z=CH)
                    )

            # ---- attention helpers ------------------------------------
            a_sb = big.tile([P, KC, T], BF16, tag="a")
            SCH_A = float(8.0 * LOG2E * EXP_SCALE)
            SCH_B = float(8.0 * (7.0 - SCH_C))

            def emit_mm1_exps(j, ew, mm2_iter, fillers):
                """MM1 + exp for pair j. Between the 64-row MM1 blocks
                (which read as ~50% PE activity to the HAM clock gate) we
                interleave full-activity work: one filler unit per sc (a
                qkv tile or dummy burst) plus MM2 groups of pair j-1 after
                every other sc. This keeps the activity monitor from
                clamping the PE clock to 1.2 GHz during the exp-paced
                stretches."""
                for sc in range(SCN):
                    ptiles = {}
                    for hb in range(2):
                        h0 = hb * CH
                        pw1 = ps1.tile([P, T], F32, tag="mm1")
                        for t in range(NT):
                            nc.tensor.matmul(
                                pw1[:, t * 512:(t + 1) * 512],
                                lhsT=k_sb[h0:h0 + CH, j, sc * P:(sc + 1) * P],
                                rhs=q_sb[h0:h0 + CH, j, t * 512:(t + 1) * 512],
                                start=True,
                                stop=True,
                            )
                        ptiles[hb] = pw1
                    scp, half = sc // 2, sc % 2
                    for hb in range(2):
                        if (scp, hb) not in ew:
                            et = ewpool.tile([P, 2, T], FP8, tag="ew")
                            ew[(scp, hb)] = et
                        et = ew[(scp, hb)]
                        if (sc, hb) in EXPS_DVE[j]:
                            nc.vector.tensor_scalar(
                                out=et[:, half, :].bitcast(I8), in0=ptiles[hb],
                                scalar1=SCH_A, scalar2=SCH_B,
                                op0=ALU.mult, op1=ALU.add,
                            )
                        else:
                            nc.scalar.activation(
                                out=et[:, half, :], in_=ptiles[hb],
                                func=AF.Exp, scale=float(EXP_SCALE),
                            )
                        if debug_taps and j == 0 and hb == 0:
                            nc.sync.dma_start(
                                out=dbg["ew0"][sc],
                                in_=et[:, half, :].bitcast(mybir.dt.uint8),
                            )
                    if fillers:
                        for _ in range(min(len(fillers), max(1, -(-len(fillers) // (SCN - sc))))):
                            fillers.popleft()()
                    # advance on even sc: group 0 then fills the pair
                    # boundary (all of pair j-1's exps are done by then)
                    if mm2_iter is not None and sc % 2 == 0:
                        next(mm2_iter, None)

            def mm2_groups(j, ew, den_on_scalar=False):
                """Generator: one MM2 group (+t-epilogue when ready) per next()."""
                pa = {}
                for t in range(NT):
                    for hb in range(2):
                        h = 2 * j + hb
                        pt = ps2.tile([P, 512], F32, tag="ps2")
                        for scp in range(SCP):
                            nc.tensor.matmul(
                                pt,
                                lhsT=vT_sb[:, 2 * scp:2 * scp + 2, h * P:(h + 1) * P],
                                rhs=ew[(scp, hb)][:, :, t * 512:(t + 1) * 512],
                                start=(scp == 0),
                                stop=(scp == SCP - 1),
                                perf_mode=DR,
                            )
                        pa[hb] = pt
                        if hb == 1:
                            # epilogue for this t: stack both denominators,
                            # one reciprocal, two normalizing muls -> a(bf16)
                            ceng = nc.scalar if den_on_scalar else nc.vector
                            dd = dnpool.tile([P, 512], F32, tag="dd")
                            if den_on_scalar:
                                ceng.copy(out=dd[0:CH, :], in_=pa[0][CH:2 * CH, :])
                                ceng.copy(out=dd[CH:P, :], in_=pa[1][CH:2 * CH, :])
                            else:
                                nc.vector.tensor_copy(out=dd[0:CH, :], in_=pa[0][CH:2 * CH, :])
                                nc.vector.tensor_copy(out=dd[CH:P, :], in_=pa[1][CH:2 * CH, :])
                            rr = dnpool.tile([P, 512], F32, tag="rr")
                            nc.vector.reciprocal_approx_fast(out=rr, in_=dd)
                            for hb2 in range(2):
                                nc.vector.tensor_mul(
                                    out=a_sb[hb2 * CH:(hb2 + 1) * CH, j,
                                             t * 512:(t + 1) * 512],
                                    in0=pa[hb2][0:CH, :],
                                    in1=rr[hb2 * CH:(hb2 + 1) * CH, :],
                                )
                        yield

            def drain(it):
                if it is not None:
                    for _ in it:
                        pass

            # ---- pipeline ---------------------------------------------
            # qk(0) upfront; v + qk(1) run as pair-0 filler, qk(2)/qk(3)
            # as pair-1 filler; pairs 2-3 get small dummy bursts. Real
            # full-activity matmul filler between the 64-row MM1 blocks is
            # what keeps the HAM clock gate at full rate.
            from collections import deque

            emit_dummies(10)  # bridge the GN-tail wait for xn
            emit_qk(0)
            fill = {
                0: deque([lambda sc=sc: emit_v_tile(sc) for sc in range(SCN)]
                         + [lambda w=w, t=t: emit_qk_tile(1, w, t)
                            for w in ("q", "k") for t in range(NT)]),
                1: deque([lambda w=w, t=t: emit_qk_tile(2, w, t)
                          for w in ("q", "k") for t in range(NT)]
                         + [lambda: emit_dummies(1)] * 4),
                2: deque([lambda w=w, t=t: emit_qk_tile(3, w, t)
                          for w in ("q", "k") for t in range(NT)]
                         + [lambda: emit_dummies(1)] * 4),
                3: deque([lambda: emit_dummies(1)] * 6),
            }
            mm2_prev = None
            for j in range(KC):
                ew_cur = {}
                emit_mm1_exps(j, ew_cur, mm2_prev, fill[j])
                drain(mm2_prev)
                if j == KC - 1 and N_DUMMY_TAIL:
                    # hold the HAM clock gate open while pair-3 exps drain
                    emit_dummies(N_DUMMY_TAIL)
                mm2_prev = mm2_groups(j, ew_cur, den_on_scalar=(j == KC - 1))
            drain(mm2_prev)

            if debug_taps:
                nc.sync.dma_start(out=dbg["xn"], in_=xn_sb.bitcast(mybir.dt.uint8))
                nc.sync.dma_start(out=dbg["q"], in_=q_sb)
                nc.sync.dma_start(out=dbg["k"], in_=k_sb)
                nc.sync.dma_start(out=dbg["vt"], in_=vT_sb.bitcast(mybir.dt.uint8))
                nc.sync.dma_start(out=dbg["a"], in_=a_sb)

            # ---- output projection + residual (bf16 matmul) -----------
            # t-major: a(pair3, t0) lands before t1, so the t0 projections
            # overlap the t1 MM2/epilogue drain
            out_sb = big.tile([P, KC, T], F32, tag="osb")
            for t in range(NT):
                for o in range(KC):
                    ph = ps2.tile([P, 512], F32, tag="ps2")
                    for k in range(KC):
                        nc.tensor.matmul(
                            ph,
                            lhsT=pw_sb[:, k, o * P:(o + 1) * P],
                            rhs=a_sb[:, k, t * 512:(t + 1) * 512],
                            start=(k == 0),
                            stop=(k == KC - 1),
                        )
                    if "pb" in bias_aps:
                        nc.vector.tensor_scalar(
                            out=ph, in0=ph, scalar1=bias_aps["pb"][:, o:o + 1],
                            scalar2=None, op0=ALU.add,
                        )
                    # out = x * (1/sqrt2) + h'   (1/sqrt2 folded into pwT/pb)
                    nc.vector.scalar_tensor_tensor(
                        out=out_sb[:, o, t * 512:(t + 1) * 512],
                        in0=x_sb[:, o, t * 512:(t + 1) * 512],
                        scalar=ISQ2,
                        in1=ph,
                        op0=ALU.mult,
                        op1=ALU.add,
                    )
                    # stream each half-chunk out immediately; alternate
                    # issue queues so the tail isn't issue-serialized
                    oeng = nc.sync if t == 0 else nc.scalar
                    oeng.dma_start(
                        out=out_d.rearrange("(o p) t -> p o t", p=P)[
                            :, o, t * 512:(t + 1) * 512],
                        in_=out_sb[:, o, t * 512:(t + 1) * 512],
                    )

    nc.compile()
    return nc


def _host_prep(qkv_w, qkv_b, proj_w, proj_b):
    """Build the replicated (per-core-identical) weight/const arrays."""
    qkv_w = np.asarray(qkv_w, np.float32)
    qkv_b = np.asarray(qkv_b, np.float32)
    proj_w = np.asarray(proj_w, np.float32)
    proj_b = np.asarray(proj_b, np.float32)

    w3 = qkv_w.reshape(NH, 3 * CH, C)  # per head: rows 0:64 q, 64:128 k, 128:192 v
    b3 = qkv_b.reshape(NH, 3 * CH)
    wq = w3[:, 0:CH, :] * LAM               # [NH, CH, C]
    wk = w3[:, CH:2 * CH, :] * LAM
    wv = w3[:, 2 * CH:3 * CH, :] * LAM
    qb = (b3[:, 0:CH] * LAM).reshape(C)
    kb = (b3[:, CH:2 * CH] * LAM).reshape(C)
    vb = (b3[:, 2 * CH:3 * CH] * LAM).reshape(C)

    FP8NP = ml_dtypes.float8_e4m3
    wqT = np.ascontiguousarray(wq.reshape(C, C).T.astype(FP8NP))  # [C_in, NH*CH]
    wkT = np.ascontiguousarray(wk.reshape(C, C).T.astype(FP8NP))
    wvT = np.ascontiguousarray(wv.reshape(C, C).T.astype(FP8NP))
    pwT = np.ascontiguousarray((proj_w * ISQ2).T.astype(ml_dtypes.bfloat16))
    pb = proj_b * ISQ2

    ind16 = np.zeros((C, G), np.float32)
    ind16[np.arange(C), np.arange(C) // GS] = 1.0 / GS
    # indT rows hold the HALF-LOCAL group index (group % 16): the GroupNorm
    # broadcast runs per chunk-pair half with group stats in partitions
    # 0:16, so every chunk's indicator lives in rows 0:16.
    indT = np.zeros((G, C), np.float32)
    indT[(np.arange(C) // GS) % (G // 2), np.arange(C)] = 1.0

    return dict(
        wqT=wqT, wkT=wkT, wvT=wvT, pwT=pwT,
        qb=qb, kb=kb, vb=vb, pb=pb,
        ind16=ind16, indT=indT,
    )


def kernel(**inputs):
    x = np.asarray(inputs["x"], np.float32)
    gn_w = np.asarray(inputs["gn_w"], np.float32)
    gn_b = np.asarray(inputs["gn_b"], np.float32)
    qkv_b = np.asarray(inputs["qkv_b"], np.float32)
    proj_b = np.asarray(inputs["proj_b"], np.float32)

    prep = _host_prep(inputs["qkv_w"], qkv_b, inputs["proj_w"], proj_b)
    qkv_bias_nz = bool(np.any(qkv_b != 0))
    proj_bias_nz = bool(np.any(proj_b != 0))

    key = (qkv_bias_nz, proj_bias_nz)
    if key not in _GRAPH_CACHE:
        _GRAPH_CACHE[key] = _build_graph(qkv_bias_nz, proj_bias_nz)
    nc = _GRAPH_CACHE[key]

    shared = dict(
        wqT=prep["wqT"], wkT=prep["wkT"], wvT=prep["wvT"], pwT=prep["pwT"],
        gnw=gn_w, gnb=gn_b, ind16=prep["ind16"], indT=prep["indT"],
    )
    if qkv_bias_nz:
        shared.update(qb=prep["qb"], kb=prep["kb"], vb=prep["vb"])
    if proj_bias_nz:
        shared.update(pb=prep["pb"])

    in_maps = [
        {**shared, "x": np.ascontiguousarray(x[i].reshape(C, T))}
        for i in range(NCORES)
    ]
    res = run_bass_kernel_spmd(nc, in_maps, core_ids=list(range(NCORES)))
    out = np.stack(
        [res.results[i]["out"].reshape(C, 32, 32) for i in range(NCORES)]
    )
    kernel._last_results = res
    return out


# revision 64
# speedup vs baseline: 1.0208x; 1.0208x over previous
"""Trainium2 Bass kernel for nn_AttentionBlock_9792525435528.

Reference computation (per batch element b):
    xf = x[b].reshape(C, T)                      # C=512, T=32*32=1024
    GroupNorm(G=32) -> xn
    qkv = qkv_w @ xn + qkv_b                     # [3C, T]
    per head h (NH=8, ch=64): q,k,v; w = softmax((q*s)^T (k*s)); a = v @ w^T
    h = proj_w @ a + proj_b
    out = (xf + h) / sqrt(2)

Sharding: data-parallel over batch. 8 batch elements -> 8 NeuronCores, one
each. Weights replicated. No cross-core communication needed.

Device algorithm (fp8 + pipelined):
  - QKV and attention-MM2 run in fp8e4 with DoubleRow perf mode (2 rows of
    the contraction per PE cell -> half the matmul instructions). q/k stay
    bf16 for MM1 (64-row contraction cannot pack); proj stays bf16 for
    accuracy (it feeds the residual directly).
  - GroupNorm statistics stream behind the x DMA chunk-by-chunk; rstd uses
    the 0x5f3759df magic-constant rsqrt + a Newton step entirely on
    VectorE, so no ScalarE table set other than exp is ever loaded and the
    critical path never hops engines.
  - x streams in as eight half-chunk DMAs round-robined over the three
    DMA-capable queues (Sync/ScalarE/GpSimd); constants + weights issue on
    GpSimd behind the vT memset so they don't steal HBM bandwidth from x.
  - A burst of (nonzero-data) dummy matmuls at t=0 burns the HAM clock
    gate's cold window; v and the later qk projections are spread through
    the MM1 stretches of pairs 0-1 as full-activity filler so the
    activity monitor keeps the PE at 2.4 GHz.
  - Attention softmax: w^T[s,t] layout; exp tiles are split between ScalarE
    (table exp -> fp8 out) and VectorE (Schraudolph bit-trick exp: one
    tensor_scalar writing int8 bits of fp8e4). The softmax denominator
    comes for free from 64 lambda-columns appended to v^T in MM2 (the fp8
    v-scale lambda cancels in the num/den ratio).
  - MM2 epilogue: both half-pair denominators stack into one [128,512]
    tile, one reciprocal_approx_fast per t-chunk, two normalizing muls.
  - Weight scale lambda=16 on wq/wk/wv keeps fp8 weights out of the
    subnormal range; the q*k logit scale (QK_SCALE^2 * lambda^-2 = 1/2048)
    is folded into the exp's affine pre-scale.
"""

import ml_dtypes
import numpy as np

import concourse.bass as bass
import concourse.mybir as mybir
import concourse.tile as tile
from concourse import bacc
from concourse.bass_utils import run_bass_kernel_spmd

B, C, T = 8, 512, 1024
NH, CH, G = 8, 64, 32
GS = C // G  # 16 channels per group
EPS = 1e-6
NCORES = 8
P = 128
KC = C // P   # 4 chunks of 128 channels
SCN = T // P  # 8 s-chunks
SCP = SCN // 2  # 4 s-chunk pairs (DoubleRow)
NT = T // 512   # 2 t-chunks of 512
ISQ2 = float(1.0 / np.sqrt(2.0))
QK_SCALE2 = float(1.0 / np.sqrt(CH))  # (1/sqrt(sqrt(ch)))^2
LAM = 16.0  # fp8 scale for wq/wk/wv (keeps weights out of subnormals)
EXP_SCALE = QK_SCALE2 / (LAM * LAM)   # = 1/2048
LOG2E = 1.4426950408889634
SCH_C = 0.05  # Schraudolph bias correction (device cast = round-to-nearest)
RSQRT_MAGIC = 0x5F3759DF

F32 = mybir.dt.float32
BF16 = mybir.dt.bfloat16
FP8 = mybir.dt.float8e4
I8 = mybir.dt.int8
I16 = mybir.dt.int16
I32 = mybir.dt.int32

N_DUMMY = 16    # HAM warm-up matmuls at t=0
N_DUMMY_TAIL = 0   # tail runs throttled regardless; dummies there cost 2x
# exp tiles handed to VectorE (Schraudolph) per pair; rest go to ScalarE.
# The last pair gets more DVE so the tail drains on both engines.
EXPS_DVE = {
    0: {(1, 1), (2, 1), (3, 1), (5, 1), (6, 1), (7, 1)},
    1: {(1, 1), (3, 1), (4, 1), (5, 1), (7, 1)},
    2: {(1, 1), (3, 1), (4, 1), (5, 1), (7, 1)},
    3: {(0, 1), (1, 1), (3, 1), (4, 1), (5, 1), (7, 1)},
}

_GRAPH_CACHE = {}


def _build_graph(qkv_bias_nz: bool, proj_bias_nz: bool, debug_taps: bool = False):
    nc = bacc.Bacc("TRN2", target_bir_lowering=False, debug=False)
    AF = mybir.ActivationFunctionType
    ALU = mybir.AluOpType
    DR = mybir.MatmulPerfMode.DoubleRow

    # ---- DRAM I/O ------------------------------------------------------
    x_d = nc.dram_tensor("x", [C, T], F32, kind="ExternalInput").ap()
    wq_d = nc.dram_tensor("wqT", [C, C], FP8, kind="ExternalInput").ap()
    wk_d = nc.dram_tensor("wkT", [C, C], FP8, kind="ExternalInput").ap()
    wv_d = nc.dram_tensor("wvT", [C, C], FP8, kind="ExternalInput").ap()
    pw_d = nc.dram_tensor("pwT", [C, C], BF16, kind="ExternalInput").ap()
    gnw_d = nc.dram_tensor("gnw", [C], F32, kind="ExternalInput").ap()
    gnb_d = nc.dram_tensor("gnb", [C], F32, kind="ExternalInput").ap()
    ind16_d = nc.dram_tensor("ind16", [C, G], F32, kind="ExternalInput").ap()
    indT_d = nc.dram_tensor("indT", [G, C], F32, kind="ExternalInput").ap()
    qb_d = kb_d = vb_d = pb_d = None
    if qkv_bias_nz:
        qb_d = nc.dram_tensor("qb", [C], F32, kind="ExternalInput").ap()
        kb_d = nc.dram_tensor("kb", [C], F32, kind="ExternalInput").ap()
        vb_d = nc.dram_tensor("vb", [C], F32, kind="ExternalInput").ap()
    if proj_bias_nz:
        pb_d = nc.dram_tensor("pb", [C], F32, kind="ExternalInput").ap()
    out_d = nc.dram_tensor("out", [C, T], F32, kind="ExternalOutput").ap()
    dbg = {}
    if debug_taps:
        dbg["xn"] = nc.dram_tensor("dbg_xn", [P, KC, T], mybir.dt.uint8, kind="ExternalOutput").ap()
        dbg["q"] = nc.dram_tensor("dbg_q", [P, KC, T], BF16, kind="ExternalOutput").ap()
        dbg["k"] = nc.dram_tensor("dbg_k", [P, KC, T], BF16, kind="ExternalOutput").ap()
        dbg["vt"] = nc.dram_tensor("dbg_vt", [P, SCN, NH * P], mybir.dt.uint8, kind="ExternalOutput").ap()
        dbg["ew0"] = nc.dram_tensor("dbg_ew0", [SCN, P, T], mybir.dt.uint8, kind="ExternalOutput").ap()
        dbg["a"] = nc.dram_tensor("dbg_a", [P, KC, T], BF16, kind="ExternalOutput").ap()

    with tile.TileContext(nc) as tc:
        with (
            tc.tile_pool(name="big", bufs=1) as big,
            tc.tile_pool(name="wpool", bufs=1) as wpool,
            tc.tile_pool(name="small", bufs=1) as small,
            tc.tile_pool(name="ew", bufs=16) as ewpool,
            tc.tile_pool(name="dn", bufs=2) as dnpool,
            tc.tile_pool(name="ps1", bufs=2, space="PSUM") as ps1,
            tc.tile_pool(name="ps2", bufs=3, space="PSUM") as ps2,
            tc.tile_pool(name="psd", bufs=1, space="PSUM") as psd,
        ):
            # ---- phase 0: warmups + loads -----------------------------
            # x is the GroupNorm critical path: eight half-chunk DMAs
            # round-robin across the three DMA-capable queues (Sync,
            # ScalarE, GpSimd) so both descriptor generation and the
            # per-queue transfer bandwidth parallelize.
            x_sb = big.tile([P, KC, T], F32, tag="x")
            x_dr = x_d.rearrange("(o p) t -> p o t", p=P)
            dma_engs = (nc.sync, nc.scalar, nc.gpsimd)
            for i, (o, hlf) in enumerate((o, h) for o in range(KC) for h in range(2)):
                dma_engs[i % 3].dma_start(
                    out=x_sb[:, o, hlf * 512:(hlf + 1) * 512],
                    in_=x_dr[:, o, hlf * 512:(hlf + 1) * 512],
                )

            # Dummy matmuls burn the HAM cold window while DMAs land. All
            # dummies write one dedicated PSUM bank so they never couple
            # to the real tile rings (a dummy must never wait on another
            # engine -- its whole job is keeping the PE clock gate open).
            # non-zero, per-lane-varying data: the activity monitor the
            # clock gate feeds on appears to weight actual datapath
            # toggling, so all-zero dummies would read as idle
            dmy_sb = small.tile([P, 640], BF16, tag="dmy")
            nc.gpsimd.iota(
                dmy_sb, pattern=[[1, 640]], base=1, channel_multiplier=3,
                allow_small_or_imprecise_dtypes=True,
            )
            pdmy = psd.tile([P, 512], F32, tag="dmy")

            def emit_dummies(n):
                for _ in range(n):
                    nc.tensor.matmul(
                        pdmy, lhsT=dmy_sb[:, 0:128], rhs=dmy_sb[:, 128:640],
                        start=True, stop=True,
                    )

            emit_dummies(N_DUMMY)

            # Warm the exp table set (the only ACT_TABLE_LOAD in the kernel).
            warm = small.tile([G, 1], F32, tag="warm")
            nc.vector.memset(warm, 1.0)
            nc.scalar.activation(out=warm, in_=warm, func=AF.Exp)

            # GpSimd issues the tiny constants, then stalls on the big vT
            # memset before issuing the weight DMAs -- which keeps the
            # weight transfers from stealing HBM bandwidth from x.
            gnw_sb = small.tile([P, KC], F32, tag="gnw")
            nc.gpsimd.dma_start(out=gnw_sb, in_=gnw_d.rearrange("(o p) -> p o", p=P))
            gnb_sb = small.tile([P, KC], F32, tag="gnb")
            nc.gpsimd.dma_start(out=gnb_sb, in_=gnb_d.rearrange("(o p) -> p o", p=P))
            ind16_sb = small.tile([P, KC, G], F32, tag="ind16")
            nc.gpsimd.dma_start(
                out=ind16_sb, in_=ind16_d.rearrange("(o p) g -> p o g", p=P)
            )
            indT_sb = small.tile([G, KC, P], F32, tag="indT")
            nc.gpsimd.dma_start(out=indT_sb, in_=indT_d.rearrange("g (o p) -> g o p", p=P))

            # v^T augmented: per head 128 cols = [64 v^T cols | 64 lambda].
            vT_sb = big.tile([P, SCN, NH * P], FP8, tag="vT")
            vT4 = vT_sb.rearrange("p s (h z) -> p s h z", z=P)
            nc.gpsimd.memset(vT4[:, :, :, CH:P], LAM)

            wq_sb = wpool.tile([P, KC, C], FP8, tag="wq")
            nc.gpsimd.dma_start(out=wq_sb, in_=wq_d.rearrange("(o p) n -> p o n", p=P))
            wk_sb = wpool.tile([P, KC, C], FP8, tag="wk")
            nc.gpsimd.dma_start(out=wk_sb, in_=wk_d.rearrange("(o p) n -> p o n", p=P))
            wv_sb = wpool.tile([P, KC, C], FP8, tag="wv")
            nc.gpsimd.dma_start(out=wv_sb, in_=wv_d.rearrange("(o p) n -> p o n", p=P))
            pw_sb = wpool.tile([P, KC, C], BF16, tag="pw")
            nc.gpsimd.dma_start(out=pw_sb, in_=pw_d.rearrange("(o p) n -> p o n", p=P))

            bias_aps = {}
            for nm, d in (("qb", qb_d), ("kb", kb_d), ("pb", pb_d)):
                if d is not None:
                    t_ = small.tile([P, KC], F32, tag=nm)
                    nc.gpsimd.dma_start(out=t_, in_=d.rearrange("(o p) -> p o", p=P))
                    bias_aps[nm] = t_
            if vb_d is not None:
                vb_bc = small.tile([P, C], F32, tag="vb")
                nc.gpsimd.dma_start(
                    out=vb_bc,
                    in_=bass.AP(tensor=vb_d.tensor, offset=vb_d.offset,
                                ap=[[0, P]] + vb_d.ap),
                )
                bias_aps["vb"] = vb_bc

            # ---- phase 1: GroupNorm (streams behind the x DMA) --------
            # GroupNorm groups (16 consecutive channels) never cross the
            # 128-channel chunks, so the whole normalization splits into
            # two independent half-pipelines over chunk pairs (0,1) and
            # (2,3): each half finishes its stats -> rsqrt -> broadcast ->
            # xn as soon as its own chunks land, instead of waiting for
            # all of x.
            stats6 = small.tile([P, KC, 2, 6], F32, tag="stats6")
            stats2 = small.tile([P, KC, 2], F32, tag="stats2")
            mtmp = small.tile([P, KC], F32, tag="mtmp")
            A_sb = small.tile([P, KC], F32, tag="A")
            B_sb = small.tile([P, KC], F32, tag="B")
            xn_sb = big.tile([P, KC, T], FP8, tag="xn")
            GH = G // 2  # 16 groups per half
            for half in range(2):
                for o in (2 * half, 2 * half + 1):
                    for hlf in range(2):
                        nc.vector.bn_stats(
                            out=stats6[:, o, hlf, :],
                            in_=x_sb[:, o, hlf * 512:(hlf + 1) * 512],
                        )
                    # stats2 = (mean, E[x^2]): aggr writes (mean, var) in
                    # place, then var += mean^2
                    nc.vector.bn_aggr(out=stats2[:, o, :], in_=stats6[:, o, :, :])
                    nc.vector.tensor_mul(
                        out=mtmp[:, o:o + 1], in0=stats2[:, o, 0:1],
                        in1=stats2[:, o, 0:1],
                    )
                    nc.vector.tensor_add(
                        out=stats2[:, o, 1:2], in0=stats2[:, o, 1:2],
                        in1=mtmp[:, o:o + 1],
                    )
                psum_s = ps2.tile([GH, 2], F32, tag="ps2", name=f"psum_s{half}")
                for o in (2 * half, 2 * half + 1):
                    nc.tensor.matmul(
                        psum_s,
                        lhsT=ind16_sb[:, o, half * GH:(half + 1) * GH],
                        rhs=stats2[:, o, :],
                        start=(o == 2 * half),
                        stop=(o == 2 * half + 1),
                    )

                musd = small.tile([GH, 2], F32, tag=f"musd{half}", name=f"musd{half}")
                nc.vector.tensor_copy(out=musd, in_=psum_s)  # (mu, E[x^2])
                vg = small.tile([GH, 5], F32, tag=f"vg{half}", name=f"vg{half}")
                x_c, y_c, a_c, b_c, c_c = (vg[:, i:i + 1] for i in range(5))
                nc.vector.tensor_mul(out=x_c, in0=musd[:, 0:1], in1=musd[:, 0:1])
                nc.vector.tensor_sub(out=x_c, in0=musd[:, 1:2], in1=x_c)
                nc.vector.tensor_scalar(
                    out=x_c, in0=x_c, scalar1=EPS, scalar2=None, op0=ALU.add
                )  # x = var + eps
                nc.vector.tensor_scalar(
                    out=y_c.bitcast(I32), in0=x_c.bitcast(I32),
                    scalar1=1, scalar2=None, op0=ALU.logical_shift_right,
                )
                nc.vector.tensor_scalar(
                    out=y_c.bitcast(I32), in0=y_c.bitcast(I32),
                    scalar1=-1, scalar2=RSQRT_MAGIC,
                    op0=ALU.mult, op1=ALU.add,
                )  # y0 = magic - (bits >> 1)
                # one Newton step: rstd accurate to ~0.2% (xn-path only)
                nc.vector.tensor_mul(out=a_c, in0=x_c, in1=y_c)
                nc.vector.tensor_mul(out=b_c, in0=a_c, in1=y_c)
                nc.vector.tensor_scalar(
                    out=c_c, in0=b_c, scalar1=-0.5, scalar2=1.5,
                    op0=ALU.mult, op1=ALU.add,
                )
                nc.vector.tensor_mul(out=musd[:, 1:2], in0=y_c, in1=c_c)

                # broadcast (mu, rstd) to this half's channels: indT rows
                # for half-1 groups are copied into partitions 0:16 of
                # indT_sb at load (see indT host layout), so lhsT/rhs base
                # partitions always match.
                psum_b = ps2.tile([P, 4], F32, tag="ps2", name=f"psum_b{half}")
                for oi, o in enumerate((2 * half, 2 * half + 1)):
                    nc.tensor.matmul(
                        psum_b[:, oi * 2:(oi + 1) * 2],
                        lhsT=indT_sb[0:GH, 2 * half + oi, :], rhs=musd,
                        start=True, stop=True,
                    )
                musd_c = small.tile([P, 2, 2], F32, tag=f"musd_c{half}",
                                    name=f"musd_c{half}")
                nc.vector.tensor_copy(
                    out=musd_c, in_=psum_b.rearrange("p (o c) -> p o c", c=2)
                )

                # A = rstd * gn_w ; B = gn_b - mu * A for this half
                osl = slice(2 * half, 2 * half + 2)
                nc.vector.tensor_mul(out=A_sb[:, osl], in0=musd_c[:, :, 1],
                                     in1=gnw_sb[:, osl])
                nc.vector.tensor_mul(out=B_sb[:, osl], in0=musd_c[:, :, 0],
                                     in1=A_sb[:, osl])
                nc.vector.tensor_sub(out=B_sb[:, osl], in0=gnb_sb[:, osl],
                                     in1=B_sb[:, osl])

                # xn = x*A + B -> fp8; first chunk of the half on ScalarE,
                # second on VectorE
                for oi, o in enumerate((2 * half, 2 * half + 1)):
                    if oi == 0:
                        nc.scalar.activation(
                            out=xn_sb[:, o, :], in_=x_sb[:, o, :],
                            func=AF.Identity,
                            bias=B_sb[:, o:o + 1], scale=A_sb[:, o:o + 1],
                        )
                    else:
                        nc.vector.tensor_scalar(
                            out=xn_sb[:, o, :], in0=x_sb[:, o, :],
                            scalar1=A_sb[:, o:o + 1], scalar2=B_sb[:, o:o + 1],
                            op0=ALU.mult, op1=ALU.add,
                        )

            # ---- phase 2: QKV projections (fp8 DoubleRow) -------------
            q_sb = big.tile([P, KC, T], BF16, tag="q")
            k_sb = big.tile([P, KC, T], BF16, tag="k")

            def emit_qk_tile(j, which, t):
                """One q-or-k output tile of head pair j: 2 DR matmuls + a
                PSUM->bf16 cast. ~0.5us of full-activity PE work."""
                dst, w_sb, bias = (
                    (q_sb, wq_sb, bias_aps.get("qb")) if which == "q"
                    else (k_sb, wk_sb, bias_aps.get("kb"))
                )
                pq = ps2.tile([P, 512], F32, tag="ps2")
                for kk in range(KC // 2):
                    nc.tensor.matmul(
                        pq,
                        lhsT=w_sb[:, 2 * kk:2 * kk + 2, j * P:(j + 1) * P],
                        rhs=xn_sb[:, 2 * kk:2 * kk + 2, t * 512:(t + 1) * 512],
                        start=(kk == 0),
                        stop=(kk == KC // 2 - 1),
                        perf_mode=DR,
                    )
                dslice = dst[:, j, t * 512:(t + 1) * 512]
                if bias is not None:
                    nc.vector.tensor_scalar(
                        out=dslice, in0=pq, scalar1=bias[:, j:j + 1],
                        scalar2=None, op0=ALU.add,
                    )
                else:
                    nc.vector.tensor_copy(out=dslice, in_=pq)

            def emit_qk(j):
                for which in ("q", "k"):
                    for t in range(NT):
                        emit_qk_tile(j, which, t)

            def emit_v_tile(sc):
                pv = ps2.tile([P, 512], F32, tag="ps2")
                for kk in range(KC // 2):
                    nc.tensor.matmul(
                        pv,
                        lhsT=xn_sb[:, 2 * kk:2 * kk + 2, sc * P:(sc + 1) * P],
                        rhs=wv_sb[:, 2 * kk:2 * kk + 2, :],
                        start=(kk == 0),
                        stop=(kk == KC // 2 - 1),
                        perf_mode=DR,
                    )
                vdst = vT4[:, sc, :, 0:CH]  # [P, NH, CH] strided dst
                if "vb" in bias_aps:
                    nc.vector.scalar_tensor_tensor(
                        out=vdst, in0=pv.rearrange("p (h z) -> p h z", z=CH),
                        scalar=0.0,
                        in1=bias_aps["vb"].rearrange("p (h z) -> p h z", z=CH),
                        op0=ALU.add, op1=ALU.add,
                    )
                elif sc % 2 == 0:
                    # v runs as pair-0 filler: split the PSUM->fp8 copies
                    # across both elementwise engines
                    nc.scalar.copy(
                        out=vdst, in_=pv.rearrange("p (h z) -> p h z", z=CH)
                    )
                else:
                    nc.vector.tensor_copy(
                        out=vdst, in_=pv.rearrange("p (h z) -> p h z", z=CH)
                    )

            # ---- attention helpers ------------------------------------
            a_sb = big.tile([P, KC, T], BF16, tag="a")
            SCH_A = float(8.0 * LOG2E * EXP_SCALE)
            SCH_B = float(8.0 * (7.0 - SCH_C))

            def emit_mm1_exps(j, ew, mm2_iter, fillers):
                """MM1 + exp for pair j. Between the 64-row MM1 blocks
                (which read as ~50% PE activity to the HAM clock gate) we
                interleave full-activity work: one filler unit per sc (a
                qkv tile or dummy burst) plus MM2 groups of pair j-1 after
                every other sc. This keeps the activity monitor from
                clamping the PE clock to 1.2 GHz during the exp-paced
                stretches."""
                for sc in range(SCN):
                    ptiles = {}
                    for hb in range(2):
                        h0 = hb * CH
                        pw1 = ps1.tile([P, T], F32, tag="mm1")
                        for t in range(NT):
                            nc.tensor.matmul(
                                pw1[:, t * 512:(t + 1) * 512],
                                lhsT=k_sb[h0:h0 + CH, j, sc * P:(sc + 1) * P],
                                rhs=q_sb[h0:h0 + CH, j, t * 512:(t + 1) * 512],
                                start=True,
                                stop=True,
                            )
                        ptiles[hb] = pw1
                    scp, half = sc // 2, sc % 2
                    for hb in range(2):
                        if (scp, hb) not in ew:
                            et = ewpool.tile([P, 2, T], FP8, tag="ew")
                            ew[(scp, hb)] = et
                        et = ew[(scp, hb)]
                        if (sc, hb) in EXPS_DVE[j]:
                            nc.vector.tensor_scalar(
                                out=et[:, half, :].bitcast(I8), in0=ptiles[hb],
                                scalar1=SCH_A, scalar2=SCH_B,
                                op0=ALU.mult, op1=ALU.add,
                            )
                        else:
                            nc.scalar.activation(
                                out=et[:, half, :], in_=ptiles[hb],
                                func=AF.Exp, scale=float(EXP_SCALE),
                            )
                        if debug_taps and j == 0 and hb == 0:
                            nc.sync.dma_start(
                                out=dbg["ew0"][sc],
                                in_=et[:, half, :].bitcast(mybir.dt.uint8),
                            )
                    if fillers:
                        for _ in range(min(len(fillers), max(1, -(-len(fillers) // (SCN - sc))))):
                            fillers.popleft()()
                    # advance on even sc: group 0 then fills the pair
                    # boundary (all of pair j-1's exps are done by then)
                    if mm2_iter is not None and sc % 2 == 0:
                        next(mm2_iter, None)

            def mm2_groups(j, ew, den_on_scalar=False):
                """Generator: one MM2 group (+t-epilogue when ready) per next()."""
                pa = {}
                for t in range(NT):
                    for hb in range(2):
                        h = 2 * j + hb
                        pt = ps2.tile([P, 512], F32, tag="ps2")
                        for scp in range(SCP):
                            nc.tensor.matmul(
                                pt,
                                lhsT=vT_sb[:, 2 * scp:2 * scp + 2, h * P:(h + 1) * P],
                                rhs=ew[(scp, hb)][:, :, t * 512:(t + 1) * 512],
                                start=(scp == 0),
                                stop=(scp == SCP - 1),
                                perf_mode=DR,
                            )
                        pa[hb] = pt
                        if hb == 1:
                            # epilogue for this t: stack both denominators,
                            # one reciprocal, two normalizing muls -> a(bf16)
                            ceng = nc.scalar if den_on_scalar else nc.vector
                            dd = dnpool.tile([P, 512], F32, tag="dd")
                            if den_on_scalar:
                                ceng.copy(out=dd[0:CH, :], in_=pa[0][CH:2 * CH, :])
                                ceng.copy(out=dd[CH:P, :], in_=pa[1][CH:2 * CH, :])
                            else:
                                nc.vector.tensor_copy(out=dd[0:CH, :], in_=pa[0][CH:2 * CH, :])
                                nc.vector.tensor_copy(out=dd[CH:P, :], in_=pa[1][CH:2 * CH, :])
                            rr = dnpool.tile([P, 512], F32, tag="rr")
                            nc.vector.reciprocal_approx_fast(out=rr, in_=dd)
                            for hb2 in range(2):
                                nc.vector.tensor_mul(
                                    out=a_sb[hb2 * CH:(hb2 + 1) * CH, j,
                                             t * 512:(t + 1) * 512],
                                    in0=pa[hb2][0:CH, :],
                                    in1=rr[hb2 * CH:(hb2 + 1) * CH, :],
                                )
                        yield

            def drain(it):
                if it is not None:
                    for _ in it:
                        pass

            # ---- pipeline ---------------------------------------------
            # qk(0) upfront; v + qk(1) run as pair-0 filler, qk(2)/qk(3)
            # as pair-1 filler; pairs 2-3 get small dummy bursts. Real
            # full-activity matmul filler between the 64-row MM1 blocks is
            # what keeps the HAM clock gate at full rate.
            from collections import deque

            emit_dummies(10)  # bridge the GN-tail wait for xn
            emit_qk(0)
            fill = {
                0: deque([lambda sc=sc: emit_v_tile(sc) for sc in range(SCN)]
                         + [lambda w=w, t=t: emit_qk_tile(1, w, t)
                            for w in ("q", "k") for t in range(NT)]),
                1: deque([lambda w=w, t=t: emit_qk_tile(2, w, t)
                          for w in ("q", "k") for t in range(NT)]
                         + [lambda w=w2, t=t2: emit_qk_tile(3, w, t)
                            for w2 in ("q", "k") for t2 in range(NT)]),
                2: deque([lambda: emit_dummies(1)] * 8),
                3: deque([lambda: emit_dummies(1)] * 6),
            }
            mm2_prev = None
            for j in range(KC):
                ew_cur = {}
                emit_mm1_exps(j, ew_cur, mm2_prev, fill[j])
                drain(mm2_prev)
                if j == KC - 1 and N_DUMMY_TAIL:
                    # hold the HAM clock gate open while pair-3 exps drain
                    emit_dummies(N_DUMMY_TAIL)
                mm2_prev = mm2_groups(j, ew_cur, den_on_scalar=(j == KC - 1))
            drain(mm2_prev)

            if debug_taps:
                nc.sync.dma_start(out=dbg["xn"], in_=xn_sb.bitcast(mybir.dt.uint8))
                nc.sync.dma_start(out=dbg["q"], in_=q_sb)
                nc.sync.dma_start(out=dbg["k"], in_=k_sb)
                nc.sync.dma_start(out=dbg["vt"], in_=vT_sb.bitcast(mybir.dt.uint8))
                nc.sync.dma_start(out=dbg["a"], in_=a_sb)

            # ---- output projection + residual (bf16 matmul) -----------
            # t-major: a(pair3, t0) lands before t1, so the t0 projections
            # overlap the t1 MM2/epilogue drain
            out_sb = big.tile([P, KC, T], F32, tag="osb")
            for t in range(NT):
                for o in range(KC):
                    ph = ps2.tile([P, 512], F32, tag="ps2")
                    for k in range(KC):
                        nc.tensor.matmul(
                            ph,
                            lhsT=pw_sb[:, k, o * P:(o + 1) * P],
                            rhs=a_sb[:, k, t * 512:(t + 1) * 512],
                            start=(k == 0),
                            stop=(k == KC - 1),
                        )
                    if "pb" in bias_aps:
                        nc.vector.tensor_scalar(
                            out=ph, in0=ph, scalar1=bias_aps["pb"][:, o:o + 1],
                            scalar2=None, op0=ALU.add,
                        )
                    # out = x * (1/sqrt2) + h'   (1/sqrt2 folded into pwT/pb)
                    nc.vector.scalar_tensor_tensor(
                        out=out_sb[:, o, t * 512:(t + 1) * 512],
                        in0=x_sb[:, o, t * 512:(t + 1) * 512],
                        scalar=ISQ2,
                        in1=ph,
                        op0=ALU.mult,
                        op1=ALU.add,
                    )
                    # stream each half-chunk out immediately; alternate
                    # issue queues so the tail isn't issue-serialized
                    oeng = nc.sync if t == 0 else nc.scalar
                    oeng.dma_start(
                        out=out_d.rearrange("(o p) t -> p o t", p=P)[
                            :, o, t * 512:(t + 1) * 512],
                        in_=out_sb[:, o, t * 512:(t + 1) * 512],
                    )

    nc.compile()
    return nc


def _host_prep(qkv_w, qkv_b, proj_w, proj_b):
    """Build the replicated (per-core-identical) weight/const arrays."""
    qkv_w = np.asarray(qkv_w, np.float32)
    qkv_b = np.asarray(qkv_b, np.float32)
    proj_w = np.asarray(proj_w, np.float32)
    proj_b = np.asarray(proj_b, np.float32)

    w3 = qkv_w.reshape(NH, 3 * CH, C)  # per head: rows 0:64 q, 64:128 k, 128:192 v
    b3 = qkv_b.reshape(NH, 3 * CH)
    wq = w3[:, 0:CH, :] * LAM               # [NH, CH, C]
    wk = w3[:, CH:2 * CH, :] * LAM
    wv = w3[:, 2 * CH:3 * CH, :] * LAM
    qb = (b3[:, 0:CH] * LAM).reshape(C)
    kb = (b3[:, CH:2 * CH] * LAM).reshape(C)
    vb = (b3[:, 2 * CH:3 * CH] * LAM).reshape(C)

    FP8NP = ml_dtypes.float8_e4m3
    wqT = np.ascontiguousarray(wq.reshape(C, C).T.astype(FP8NP))  # [C_in, NH*CH]
    wkT = np.ascontiguousarray(wk.reshape(C, C).T.astype(FP8NP))
    wvT = np.ascontiguousarray(wv.reshape(C, C).T.astype(FP8NP))
    pwT = np.ascontiguousarray((proj_w * ISQ2).T.astype(ml_dtypes.bfloat16))
    pb = proj_b * ISQ2

    ind16 = np.zeros((C, G), np.float32)
    ind16[np.arange(C), np.arange(C) // GS] = 1.0 / GS
    # indT rows hold the HALF-LOCAL group index (group % 16): the GroupNorm
    # broadcast runs per chunk-pair half with group stats in partitions
    # 0:16, so every chunk's indicator lives in rows 0:16.
    indT = np.zeros((G, C), np.float32)
    indT[(np.arange(C) // GS) % (G // 2), np.arange(C)] = 1.0

    return dict(
        wqT=wqT, wkT=wkT, wvT=wvT, pwT=pwT,
        qb=qb, kb=kb, vb=vb, pb=pb,
        ind16=ind16, indT=indT,
    )


def kernel(**inputs):
    x = np.asarray(inputs["x"], np.float32)
    gn_w = np.asarray(inputs["gn_w"], np.float32)
    gn_b = np.asarray(inputs["gn_b"], np.float32)
    qkv_b = np.asarray(inputs["qkv_b"], np.float32)
    proj_b = np.asarray(inputs["proj_b"], np.float32)

    prep = _host_prep(inputs["qkv_w"], qkv_b, inputs["proj_w"], proj_b)
    qkv_bias_nz = bool(np.any(qkv_b != 0))
    proj_bias_nz = bool(np.any(proj_b != 0))

    key = (qkv_bias_nz, proj_bias_nz)
    if key not in _GRAPH_CACHE:
        _GRAPH_CACHE[key] = _build_graph(qkv_bias_nz, proj_bias_nz)
    nc = _GRAPH_CACHE[key]

    shared = dict(
        wqT=prep["wqT"], wkT=prep["wkT"], wvT=prep["wvT"], pwT=prep["pwT"],
        gnw=gn_w, gnb=gn_b, ind16=prep["ind16"], indT=prep["indT"],
    )
    if qkv_bias_nz:
        shared.update(qb=prep["qb"], kb=prep["kb"], vb=prep["vb"])
    if proj_bias_nz:
        shared.update(pb=prep["pb"])

    in_maps = [
        {**shared, "x": np.ascontiguousarray(x[i].reshape(C, T))}
        for i in range(NCORES)
    ]
    res = run_bass_kernel_spmd(nc, in_maps, core_ids=list(range(NCORES)))
    out = np.stack(
        [res.results[i]["out"].reshape(C, 32, 32) for i in range(NCORES)]
    )
    kernel._last_results = res
    return out


# revision 65
# speedup vs baseline: 1.0732x; 1.0514x over previous
"""Trainium2 Bass kernel for nn_AttentionBlock_9792525435528.

Reference computation (per batch element b):
    xf = x[b].reshape(C, T)                      # C=512, T=32*32=1024
    GroupNorm(G=32) -> xn
    qkv = qkv_w @ xn + qkv_b                     # [3C, T]
    per head h (NH=8, ch=64): q,k,v; w = softmax((q*s)^T (k*s)); a = v @ w^T
    h = proj_w @ a + proj_b
    out = (xf + h) / sqrt(2)

Sharding: data-parallel over batch. 8 batch elements -> 8 NeuronCores, one
each. Weights replicated. No cross-core communication needed.

Device algorithm (fp8 + pipelined):
  - QKV and attention-MM2 run in fp8e4 with DoubleRow perf mode (2 rows of
    the contraction per PE cell -> half the matmul instructions). q/k stay
    bf16 for MM1 (64-row contraction cannot pack); proj stays bf16 for
    accuracy (it feeds the residual directly).
  - GroupNorm statistics stream behind the x DMA chunk-by-chunk; rstd uses
    the 0x5f3759df magic-constant rsqrt + a Newton step entirely on
    VectorE, so no ScalarE table set other than exp is ever loaded and the
    critical path never hops engines.
  - x streams in as eight half-chunk DMAs round-robined over the three
    DMA-capable queues (Sync/ScalarE/GpSimd); constants + weights issue on
    GpSimd behind the vT memset so they don't steal HBM bandwidth from x.
  - A burst of (nonzero-data) dummy matmuls at t=0 burns the HAM clock
    gate's cold window; v and the later qk projections are spread through
    the MM1 stretches of pairs 0-1 as full-activity filler so the
    activity monitor keeps the PE at 2.4 GHz.
  - Attention softmax: w^T[s,t] layout; exp tiles are split between ScalarE
    (table exp -> fp8 out) and VectorE (Schraudolph bit-trick exp: one
    tensor_scalar writing int8 bits of fp8e4). The softmax denominator
    comes for free from 64 lambda-columns appended to v^T in MM2 (the fp8
    v-scale lambda cancels in the num/den ratio).
  - MM2 epilogue: both half-pair denominators stack into one [128,512]
    tile, one reciprocal_approx_fast per t-chunk, two normalizing muls.
  - Weight scale lambda=16 on wq/wk/wv keeps fp8 weights out of the
    subnormal range; the q*k logit scale (QK_SCALE^2 * lambda^-2 = 1/2048)
    is folded into the exp's affine pre-scale.
"""

import ml_dtypes
import numpy as np

import concourse.bass as bass
import concourse.mybir as mybir
import concourse.tile as tile
from concourse import bacc
from concourse.bass_utils import run_bass_kernel_spmd

B, C, T = 8, 512, 1024
NH, CH, G = 8, 64, 32
GS = C // G  # 16 channels per group
EPS = 1e-6
NCORES = 8
P = 128
KC = C // P   # 4 chunks of 128 channels
SCN = T // P  # 8 s-chunks
SCP = SCN // 2  # 4 s-chunk pairs (DoubleRow)
NT = T // 512   # 2 t-chunks of 512
ISQ2 = float(1.0 / np.sqrt(2.0))
QK_SCALE2 = float(1.0 / np.sqrt(CH))  # (1/sqrt(sqrt(ch)))^2
LAM = 16.0  # fp8 scale for wq/wk/wv (keeps weights out of subnormals)
EXP_SCALE = QK_SCALE2 / (LAM * LAM)   # = 1/2048
LOG2E = 1.4426950408889634
SCH_C = 0.05  # Schraudolph bias correction (device cast = round-to-nearest)
RSQRT_MAGIC = 0x5F3759DF

F32 = mybir.dt.float32
BF16 = mybir.dt.bfloat16
FP8 = mybir.dt.float8e4
I8 = mybir.dt.int8
I16 = mybir.dt.int16
I32 = mybir.dt.int32

N_DUMMY = 16    # HAM warm-up matmuls at t=0
N_DUMMY_TAIL = 0   # tail runs throttled regardless; dummies there cost 2x
# exp tiles handed to VectorE (Schraudolph) per pair; rest go to ScalarE.
# The last pair gets more DVE so the tail drains on both engines.
EXPS_DVE = {
    0: {(1, 1), (2, 1), (3, 1), (5, 1), (6, 1), (7, 1)},
    1: {(1, 1), (3, 1), (4, 1), (5, 1), (7, 1)},
    2: {(1, 1), (3, 1), (5, 1), (7, 1)},
    3: {(0, 1), (1, 1), (3, 1), (4, 1), (5, 1), (7, 1)},
}

_GRAPH_CACHE = {}


def _build_graph(qkv_bias_nz: bool, proj_bias_nz: bool, debug_taps: bool = False):
    nc = bacc.Bacc("TRN2", target_bir_lowering=False, debug=False)
    AF = mybir.ActivationFunctionType
    ALU = mybir.AluOpType
    DR = mybir.MatmulPerfMode.DoubleRow

    # ---- DRAM I/O ------------------------------------------------------
    x_d = nc.dram_tensor("x", [C, T], F32, kind="ExternalInput").ap()
    wq_d = nc.dram_tensor("wqT", [C, C], FP8, kind="ExternalInput").ap()
    wk_d = nc.dram_tensor("wkT", [C, C], FP8, kind="ExternalInput").ap()
    wv_d = nc.dram_tensor("wvT", [C, C], FP8, kind="ExternalInput").ap()
    pw_d = nc.dram_tensor("pwT", [C, C], BF16, kind="ExternalInput").ap()
    gnw_d = nc.dram_tensor("gnw", [C], F32, kind="ExternalInput").ap()
    gnb_d = nc.dram_tensor("gnb", [C], F32, kind="ExternalInput").ap()
    ind16_d = nc.dram_tensor("ind16", [C, G], F32, kind="ExternalInput").ap()
    indT_d = nc.dram_tensor("indT", [G, C], F32, kind="ExternalInput").ap()
    qb_d = kb_d = vb_d = pb_d = None
    if qkv_bias_nz:
        qb_d = nc.dram_tensor("qb", [C], F32, kind="ExternalInput").ap()
        kb_d = nc.dram_tensor("kb", [C], F32, kind="ExternalInput").ap()
        vb_d = nc.dram_tensor("vb", [C], F32, kind="ExternalInput").ap()
    if proj_bias_nz:
        pb_d = nc.dram_tensor("pb", [C], F32, kind="ExternalInput").ap()
    out_d = nc.dram_tensor("out", [C, T], F32, kind="ExternalOutput").ap()
    dbg = {}
    if debug_taps:
        dbg["xn"] = nc.dram_tensor("dbg_xn", [P, KC, T], mybir.dt.uint8, kind="ExternalOutput").ap()
        dbg["q"] = nc.dram_tensor("dbg_q", [P, KC, T], BF16, kind="ExternalOutput").ap()
        dbg["k"] = nc.dram_tensor("dbg_k", [P, KC, T], BF16, kind="ExternalOutput").ap()
        dbg["vt"] = nc.dram_tensor("dbg_vt", [P, SCN, NH * P], mybir.dt.uint8, kind="ExternalOutput").ap()
        dbg["ew0"] = nc.dram_tensor("dbg_ew0", [SCN, P, T], mybir.dt.uint8, kind="ExternalOutput").ap()
        dbg["a"] = nc.dram_tensor("dbg_a", [P, KC, T], BF16, kind="ExternalOutput").ap()

    with tile.TileContext(nc) as tc:
        with (
            tc.tile_pool(name="big", bufs=1) as big,
            tc.tile_pool(name="wpool", bufs=1) as wpool,
            tc.tile_pool(name="small", bufs=1) as small,
            tc.tile_pool(name="ew", bufs=16) as ewpool,
            tc.tile_pool(name="dn", bufs=2) as dnpool,
            tc.tile_pool(name="ps1", bufs=2, space="PSUM") as ps1,
            tc.tile_pool(name="ps2", bufs=3, space="PSUM") as ps2,
            tc.tile_pool(name="psd", bufs=1, space="PSUM") as psd,
        ):
            # ---- phase 0: warmups + loads -----------------------------
            # x is the GroupNorm critical path: eight half-chunk DMAs
            # round-robin across the three DMA-capable queues (Sync,
            # ScalarE, GpSimd) so both descriptor generation and the
            # per-queue transfer bandwidth parallelize.
            x_sb = big.tile([P, KC, T], F32, tag="x")
            x_dr = x_d.rearrange("(o p) t -> p o t", p=P)
            dma_engs = (nc.sync, nc.scalar, nc.gpsimd)
            for i, (o, hlf) in enumerate((o, h) for o in range(KC) for h in range(2)):
                dma_engs[i % 3].dma_start(
                    out=x_sb[:, o, hlf * 512:(hlf + 1) * 512],
                    in_=x_dr[:, o, hlf * 512:(hlf + 1) * 512],
                )

            # Dummy matmuls burn the HAM cold window while DMAs land. All
            # dummies write one dedicated PSUM bank so they never couple
            # to the real tile rings (a dummy must never wait on another
            # engine -- its whole job is keeping the PE clock gate open).
            # non-zero, per-lane-varying data: the activity monitor the
            # clock gate feeds on appears to weight actual datapath
            # toggling, so all-zero dummies would read as idle
            dmy_sb = small.tile([P, 640], BF16, tag="dmy")
            nc.gpsimd.iota(
                dmy_sb, pattern=[[1, 640]], base=1, channel_multiplier=3,
                allow_small_or_imprecise_dtypes=True,
            )
            pdmy = psd.tile([P, 512], F32, tag="dmy")

            def emit_dummies(n):
                for _ in range(n):
                    nc.tensor.matmul(
                        pdmy, lhsT=dmy_sb[:, 0:128], rhs=dmy_sb[:, 128:640],
                        start=True, stop=True,
                    )

            emit_dummies(N_DUMMY)

            # Warm the exp table set (the only ACT_TABLE_LOAD in the kernel).
            warm = small.tile([G, 1], F32, tag="warm")
            nc.vector.memset(warm, 1.0)
            nc.scalar.activation(out=warm, in_=warm, func=AF.Exp)

            # GpSimd issues the tiny constants, then stalls on the big vT
            # memset before issuing the weight DMAs -- which keeps the
            # weight transfers from stealing HBM bandwidth from x.
            gnw_sb = small.tile([P, KC], F32, tag="gnw")
            nc.gpsimd.dma_start(out=gnw_sb, in_=gnw_d.rearrange("(o p) -> p o", p=P))
            gnb_sb = small.tile([P, KC], F32, tag="gnb")
            nc.gpsimd.dma_start(out=gnb_sb, in_=gnb_d.rearrange("(o p) -> p o", p=P))
            ind16_sb = small.tile([P, KC, G], F32, tag="ind16")
            nc.gpsimd.dma_start(
                out=ind16_sb, in_=ind16_d.rearrange("(o p) g -> p o g", p=P)
            )
            indT_sb = small.tile([G, KC, P], F32, tag="indT")
            nc.gpsimd.dma_start(out=indT_sb, in_=indT_d.rearrange("g (o p) -> g o p", p=P))

            # v^T augmented: per head 128 cols = [64 v^T cols | 64 lambda].
            vT_sb = big.tile([P, SCN, NH * P], FP8, tag="vT")
            vT4 = vT_sb.rearrange("p s (h z) -> p s h z", z=P)
            nc.gpsimd.memset(vT4[:, :, :, CH:P], LAM)

            wq_sb = wpool.tile([P, KC, C], FP8, tag="wq")
            nc.gpsimd.dma_start(out=wq_sb, in_=wq_d.rearrange("(o p) n -> p o n", p=P))
            wk_sb = wpool.tile([P, KC, C], FP8, tag="wk")
            nc.gpsimd.dma_start(out=wk_sb, in_=wk_d.rearrange("(o p) n -> p o n", p=P))
            wv_sb = wpool.tile([P, KC, C], FP8, tag="wv")
            nc.gpsimd.dma_start(out=wv_sb, in_=wv_d.rearrange("(o p) n -> p o n", p=P))
            pw_sb = wpool.tile([P, KC, C], BF16, tag="pw")
            nc.gpsimd.dma_start(out=pw_sb, in_=pw_d.rearrange("(o p) n -> p o n", p=P))

            bias_aps = {}
            for nm, d in (("qb", qb_d), ("kb", kb_d), ("pb", pb_d)):
                if d is not None:
                    t_ = small.tile([P, KC], F32, tag=nm)
                    nc.gpsimd.dma_start(out=t_, in_=d.rearrange("(o p) -> p o", p=P))
                    bias_aps[nm] = t_
            if vb_d is not None:
                vb_bc = small.tile([P, C], F32, tag="vb")
                nc.gpsimd.dma_start(
                    out=vb_bc,
                    in_=bass.AP(tensor=vb_d.tensor, offset=vb_d.offset,
                                ap=[[0, P]] + vb_d.ap),
                )
                bias_aps["vb"] = vb_bc

            # ---- phase 1: GroupNorm (streams behind the x DMA) --------
            # GroupNorm groups (16 consecutive channels) never cross the
            # 128-channel chunks, so the whole normalization splits into
            # two independent half-pipelines over chunk pairs (0,1) and
            # (2,3): each half finishes its stats -> rsqrt -> broadcast ->
            # xn as soon as its own chunks land, instead of waiting for
            # all of x.
            stats6 = small.tile([P, KC, 2, 6], F32, tag="stats6")
            stats2 = small.tile([P, KC, 2], F32, tag="stats2")
            mtmp = small.tile([P, KC], F32, tag="mtmp")
            A_sb = small.tile([P, KC], F32, tag="A")
            B_sb = small.tile([P, KC], F32, tag="B")
            xn_sb = big.tile([P, KC, T], FP8, tag="xn")
            GH = G // 2  # 16 groups per half
            for half in range(2):
                for o in (2 * half, 2 * half + 1):
                    for hlf in range(2):
                        nc.vector.bn_stats(
                            out=stats6[:, o, hlf, :],
                            in_=x_sb[:, o, hlf * 512:(hlf + 1) * 512],
                        )
                    # stats2 = (mean, E[x^2]): aggr writes (mean, var) in
                    # place, then var += mean^2
                    nc.vector.bn_aggr(out=stats2[:, o, :], in_=stats6[:, o, :, :])
                    nc.vector.tensor_mul(
                        out=mtmp[:, o:o + 1], in0=stats2[:, o, 0:1],
                        in1=stats2[:, o, 0:1],
                    )
                    nc.vector.tensor_add(
                        out=stats2[:, o, 1:2], in0=stats2[:, o, 1:2],
                        in1=mtmp[:, o:o + 1],
                    )
                psum_s = ps2.tile([GH, 2], F32, tag="ps2", name=f"psum_s{half}")
                for o in (2 * half, 2 * half + 1):
                    nc.tensor.matmul(
                        psum_s,
                        lhsT=ind16_sb[:, o, half * GH:(half + 1) * GH],
                        rhs=stats2[:, o, :],
                        start=(o == 2 * half),
                        stop=(o == 2 * half + 1),
                    )

                musd = small.tile([GH, 2], F32, tag=f"musd{half}", name=f"musd{half}")
                nc.vector.tensor_copy(out=musd, in_=psum_s)  # (mu, E[x^2])
                vg = small.tile([GH, 5], F32, tag=f"vg{half}", name=f"vg{half}")
                x_c, y_c, a_c, b_c, c_c = (vg[:, i:i + 1] for i in range(5))
                nc.vector.tensor_mul(out=x_c, in0=musd[:, 0:1], in1=musd[:, 0:1])
                nc.vector.tensor_sub(out=x_c, in0=musd[:, 1:2], in1=x_c)
                nc.vector.tensor_scalar(
                    out=x_c, in0=x_c, scalar1=EPS, scalar2=None, op0=ALU.add
                )  # x = var + eps
                nc.vector.tensor_scalar(
                    out=y_c.bitcast(I32), in0=x_c.bitcast(I32),
                    scalar1=1, scalar2=None, op0=ALU.logical_shift_right,
                )
                nc.vector.tensor_scalar(
                    out=y_c.bitcast(I32), in0=y_c.bitcast(I32),
                    scalar1=-1, scalar2=RSQRT_MAGIC,
                    op0=ALU.mult, op1=ALU.add,
                )  # y0 = magic - (bits >> 1)
                # one Newton step: rstd accurate to ~0.2% (xn-path only)
                nc.vector.tensor_mul(out=a_c, in0=x_c, in1=y_c)
                nc.vector.tensor_mul(out=b_c, in0=a_c, in1=y_c)
                nc.vector.tensor_scalar(
                    out=c_c, in0=b_c, scalar1=-0.5, scalar2=1.5,
                    op0=ALU.mult, op1=ALU.add,
                )
                nc.vector.tensor_mul(out=musd[:, 1:2], in0=y_c, in1=c_c)

                # broadcast (mu, rstd) to this half's channels: indT rows
                # for half-1 groups are copied into partitions 0:16 of
                # indT_sb at load (see indT host layout), so lhsT/rhs base
                # partitions always match.
                psum_b = ps2.tile([P, 4], F32, tag="ps2", name=f"psum_b{half}")
                for oi, o in enumerate((2 * half, 2 * half + 1)):
                    nc.tensor.matmul(
                        psum_b[:, oi * 2:(oi + 1) * 2],
                        lhsT=indT_sb[0:GH, 2 * half + oi, :], rhs=musd,
                        start=True, stop=True,
                    )
                musd_c = small.tile([P, 2, 2], F32, tag=f"musd_c{half}",
                                    name=f"musd_c{half}")
                nc.vector.tensor_copy(
                    out=musd_c, in_=psum_b.rearrange("p (o c) -> p o c", c=2)
                )

                # A = rstd * gn_w ; B = gn_b - mu * A for this half
                osl = slice(2 * half, 2 * half + 2)
                nc.vector.tensor_mul(out=A_sb[:, osl], in0=musd_c[:, :, 1],
                                     in1=gnw_sb[:, osl])
                nc.vector.tensor_mul(out=B_sb[:, osl], in0=musd_c[:, :, 0],
                                     in1=A_sb[:, osl])
                nc.vector.tensor_sub(out=B_sb[:, osl], in0=gnb_sb[:, osl],
                                     in1=B_sb[:, osl])

                # xn = x*A + B -> fp8; first chunk of the half on ScalarE,
                # second on VectorE
                for oi, o in enumerate((2 * half, 2 * half + 1)):
                    if oi == 0:
                        nc.scalar.activation(
                            out=xn_sb[:, o, :], in_=x_sb[:, o, :],
                            func=AF.Identity,
                            bias=B_sb[:, o:o + 1], scale=A_sb[:, o:o + 1],
                        )
                    else:
                        nc.vector.tensor_scalar(
                            out=xn_sb[:, o, :], in0=x_sb[:, o, :],
                            scalar1=A_sb[:, o:o + 1], scalar2=B_sb[:, o:o + 1],
                            op0=ALU.mult, op1=ALU.add,
                        )

            # ---- phase 2: QKV projections (fp8 DoubleRow) -------------
            q_sb = big.tile([P, KC, T], BF16, tag="q")
            k_sb = big.tile([P, KC, T], BF16, tag="k")

            def emit_qk_tile(j, which, t):
                """One q-or-k output tile of head pair j: 2 DR matmuls + a
                PSUM->bf16 cast. ~0.5us of full-activity PE work."""
                dst, w_sb, bias = (
                    (q_sb, wq_sb, bias_aps.get("qb")) if which == "q"
                    else (k_sb, wk_sb, bias_aps.get("kb"))
                )
                pq = ps2.tile([P, 512], F32, tag="ps2")
                for kk in range(KC // 2):
                    nc.tensor.matmul(
                        pq,
                        lhsT=w_sb[:, 2 * kk:2 * kk + 2, j * P:(j + 1) * P],
                        rhs=xn_sb[:, 2 * kk:2 * kk + 2, t * 512:(t + 1) * 512],
                        start=(kk == 0),
                        stop=(kk == KC // 2 - 1),
                        perf_mode=DR,
                    )
                dslice = dst[:, j, t * 512:(t + 1) * 512]
                if bias is not None:
                    nc.vector.tensor_scalar(
                        out=dslice, in0=pq, scalar1=bias[:, j:j + 1],
                        scalar2=None, op0=ALU.add,
                    )
                else:
                    nc.vector.tensor_copy(out=dslice, in_=pq)

            def emit_qk(j):
                for which in ("q", "k"):
                    for t in range(NT):
                        emit_qk_tile(j, which, t)

            def emit_v_tile(sc, hh):
                """v-GEMM for heads 4*hh..4*hh+3 at s-chunk sc. The back
                half (hh=1) is only consumed by MM2(2)/MM2(3) in pair 3,
                so it can run as real full-activity filler inside pair 2."""
                pv = ps2.tile([P, 256], F32, tag="ps2")
                for kk in range(KC // 2):
                    nc.tensor.matmul(
                        pv,
                        lhsT=xn_sb[:, 2 * kk:2 * kk + 2, sc * P:(sc + 1) * P],
                        rhs=wv_sb[:, 2 * kk:2 * kk + 2, hh * 256:(hh + 1) * 256],
                        start=(kk == 0),
                        stop=(kk == KC // 2 - 1),
                        perf_mode=DR,
                    )
                vdst = vT4[:, sc, 4 * hh:4 * hh + 4, 0:CH]
                if "vb" in bias_aps:
                    nc.vector.scalar_tensor_tensor(
                        out=vdst, in0=pv.rearrange("p (h z) -> p h z", z=CH),
                        scalar=0.0,
                        in1=bias_aps["vb"][:, hh * 256:(hh + 1) * 256]
                            .rearrange("p (h z) -> p h z", z=CH),
                        op0=ALU.add, op1=ALU.add,
                    )
                elif sc % 2 == 0:
                    # split the PSUM->fp8 copies across both engines
                    nc.scalar.copy(
                        out=vdst, in_=pv.rearrange("p (h z) -> p h z", z=CH)
                    )
                else:
                    nc.vector.tensor_copy(
                        out=vdst, in_=pv.rearrange("p (h z) -> p h z", z=CH)
                    )

            # ---- attention helpers ------------------------------------
            a_sb = big.tile([P, KC, T], BF16, tag="a")
            SCH_A = float(8.0 * LOG2E * EXP_SCALE)
            SCH_B = float(8.0 * (7.0 - SCH_C))

            def emit_mm1_exps(j, ew, mm2_iter, fillers):
                """MM1 + exp for pair j. Between the 64-row MM1 blocks
                (which read as ~50% PE activity to the HAM clock gate) we
                interleave full-activity work: one filler unit per sc (a
                qkv tile or dummy burst) plus MM2 groups of pair j-1 after
                every other sc. This keeps the activity monitor from
                clamping the PE clock to 1.2 GHz during the exp-paced
                stretches."""
                for sc in range(SCN):
                    ptiles = {}
                    for hb in range(2):
                        h0 = hb * CH
                        pw1 = ps1.tile([P, T], F32, tag="mm1")
                        for t in range(NT):
                            nc.tensor.matmul(
                                pw1[:, t * 512:(t + 1) * 512],
                                lhsT=k_sb[h0:h0 + CH, j, sc * P:(sc + 1) * P],
                                rhs=q_sb[h0:h0 + CH, j, t * 512:(t + 1) * 512],
                                start=True,
                                stop=True,
                            )
                        ptiles[hb] = pw1
                    scp, half = sc // 2, sc % 2
                    for hb in range(2):
                        if (scp, hb) not in ew:
                            et = ewpool.tile([P, 2, T], FP8, tag="ew")
                            ew[(scp, hb)] = et
                        et = ew[(scp, hb)]
                        if (sc, hb) in EXPS_DVE[j]:
                            nc.vector.tensor_scalar(
                                out=et[:, half, :].bitcast(I8), in0=ptiles[hb],
                                scalar1=SCH_A, scalar2=SCH_B,
                                op0=ALU.mult, op1=ALU.add,
                            )
                        else:
                            nc.scalar.activation(
                                out=et[:, half, :], in_=ptiles[hb],
                                func=AF.Exp, scale=float(EXP_SCALE),
                            )
                        if debug_taps and j == 0 and hb == 0:
                            nc.sync.dma_start(
                                out=dbg["ew0"][sc],
                                in_=et[:, half, :].bitcast(mybir.dt.uint8),
                            )
                    if fillers:
                        for _ in range(min(len(fillers), max(1, -(-len(fillers) // (SCN - sc))))):
                            fillers.popleft()()
                    # advance on even sc: group 0 then fills the pair
                    # boundary (all of pair j-1's exps are done by then)
                    if mm2_iter is not None and sc % 2 == 0:
                        next(mm2_iter, None)

            def mm2_groups(j, ew, den_on_scalar=False):
                """Generator: one MM2 group (+t-epilogue when ready) per next()."""
                pa = {}
                for t in range(NT):
                    for hb in range(2):
                        h = 2 * j + hb
                        pt = ps2.tile([P, 512], F32, tag="ps2")
                        for scp in range(SCP):
                            nc.tensor.matmul(
                                pt,
                                lhsT=vT_sb[:, 2 * scp:2 * scp + 2, h * P:(h + 1) * P],
                                rhs=ew[(scp, hb)][:, :, t * 512:(t + 1) * 512],
                                start=(scp == 0),
                                stop=(scp == SCP - 1),
                                perf_mode=DR,
                            )
                        pa[hb] = pt
                        if hb == 1:
                            # epilogue for this t: stack both denominators,
                            # one reciprocal, two normalizing muls -> a(bf16)
                            ceng = nc.scalar if den_on_scalar else nc.vector
                            dd = dnpool.tile([P, 512], F32, tag="dd")
                            if den_on_scalar:
                                ceng.copy(out=dd[0:CH, :], in_=pa[0][CH:2 * CH, :])
                                ceng.copy(out=dd[CH:P, :], in_=pa[1][CH:2 * CH, :])
                            else:
                                nc.vector.tensor_copy(out=dd[0:CH, :], in_=pa[0][CH:2 * CH, :])
                                nc.vector.tensor_copy(out=dd[CH:P, :], in_=pa[1][CH:2 * CH, :])
                            rr = dnpool.tile([P, 512], F32, tag="rr")
                            nc.vector.reciprocal_approx_fast(out=rr, in_=dd)
                            for hb2 in range(2):
                                nc.vector.tensor_mul(
                                    out=a_sb[hb2 * CH:(hb2 + 1) * CH, j,
                                             t * 512:(t + 1) * 512],
                                    in0=pa[hb2][0:CH, :],
                                    in1=rr[hb2 * CH:(hb2 + 1) * CH, :],
                                )
                        yield

            def drain(it):
                if it is not None:
                    for _ in it:
                        pass

            # ---- pipeline ---------------------------------------------
            # qk(0) upfront; v + qk(1) run as pair-0 filler, qk(2)/qk(3)
            # as pair-1 filler; pairs 2-3 get small dummy bursts. Real
            # full-activity matmul filler between the 64-row MM1 blocks is
            # what keeps the HAM clock gate at full rate.
            from collections import deque

            emit_dummies(10)  # bridge the GN-tail wait for xn
            emit_qk(0)
            fill = {
                0: deque([lambda sc=sc: emit_v_tile(sc, 0) for sc in range(SCN)]
                         + [lambda w=w, t=t: emit_qk_tile(1, w, t)
                            for w in ("q", "k") for t in range(NT)]),
                1: deque([lambda w=w, t=t: emit_qk_tile(2, w, t)
                          for w in ("q", "k") for t in range(NT)]
                         + [lambda w=w2, t=t2: emit_qk_tile(3, w, t)
                            for w2 in ("q", "k") for t2 in range(NT)]),
                2: deque([lambda sc=sc: emit_v_tile(sc, 1) for sc in range(SCN)]),
                3: deque([lambda: emit_dummies(1)] * 6),
            }
            mm2_prev = None
            for j in range(KC):
                ew_cur = {}
                emit_mm1_exps(j, ew_cur, mm2_prev, fill[j])
                drain(mm2_prev)
                if j == KC - 1 and N_DUMMY_TAIL:
                    # hold the HAM clock gate open while pair-3 exps drain
                    emit_dummies(N_DUMMY_TAIL)
                mm2_prev = mm2_groups(j, ew_cur, den_on_scalar=(j == KC - 1))
            drain(mm2_prev)

            if debug_taps:
                nc.sync.dma_start(out=dbg["xn"], in_=xn_sb.bitcast(mybir.dt.uint8))
                nc.sync.dma_start(out=dbg["q"], in_=q_sb)
                nc.sync.dma_start(out=dbg["k"], in_=k_sb)
                nc.sync.dma_start(out=dbg["vt"], in_=vT_sb.bitcast(mybir.dt.uint8))
                nc.sync.dma_start(out=dbg["a"], in_=a_sb)

            # ---- output projection + residual (bf16 matmul) -----------
            # t-major: a(pair3, t0) lands before t1, so the t0 projections
            # overlap the t1 MM2/epilogue drain
            out_sb = big.tile([P, KC, T], F32, tag="osb")
            for t in range(NT):
                for o in range(KC):
                    ph = ps2.tile([P, 512], F32, tag="ps2")
                    for k in range(KC):
                        nc.tensor.matmul(
                            ph,
                            lhsT=pw_sb[:, k, o * P:(o + 1) * P],
                            rhs=a_sb[:, k, t * 512:(t + 1) * 512],
                            start=(k == 0),
                            stop=(k == KC - 1),
                        )
                    if "pb" in bias_aps:
                        nc.vector.tensor_scalar(
                            out=ph, in0=ph, scalar1=bias_aps["pb"][:, o:o + 1],
                            scalar2=None, op0=ALU.add,
                        )
                    # out = x * (1/sqrt2) + h'   (1/sqrt2 folded into pwT/pb)
                    nc.vector.scalar_tensor_tensor(
                        out=out_sb[:, o, t * 512:(t + 1) * 512],
                        in0=x_sb[:, o, t * 512:(t + 1) * 512],
                        scalar=ISQ2,
                        in1=ph,
                        op0=ALU.mult,
                        op1=ALU.add,
                    )
                    # stream each half-chunk out immediately; alternate
                    # issue queues so the tail isn't issue-serialized
                    oeng = nc.sync if t == 0 else nc.scalar
                    oeng.dma_start(
                        out=out_d.rearrange("(o p) t -> p o t", p=P)[
                            :, o, t * 512:(t + 1) * 512],
                        in_=out_sb[:, o, t * 512:(t + 1) * 512],
                    )

    nc.compile()
    return nc


def _host_prep(qkv_w, qkv_b, proj_w, proj_b):
    """Build the replicated (per-core-identical) weight/const arrays."""
    qkv_w = np.asarray(qkv_w, np.float32)
    qkv_b = np.asarray(qkv_b, np.float32)
    proj_w = np.asarray(proj_w, np.float32)
    proj_b = np.asarray(proj_b, np.float32)

    w3 = qkv_w.reshape(NH, 3 * CH, C)  # per head: rows 0:64 q, 64:128 k, 128:192 v
    b3 = qkv_b.reshape(NH, 3 * CH)
    wq = w3[:, 0:CH, :] * LAM               # [NH, CH, C]
    wk = w3[:, CH:2 * CH, :] * LAM
    wv = w3[:, 2 * CH:3 * CH, :] * LAM
    qb = (b3[:, 0:CH] * LAM).reshape(C)
    kb = (b3[:, CH:2 * CH] * LAM).reshape(C)
    vb = (b3[:, 2 * CH:3 * CH] * LAM).reshape(C)

    FP8NP = ml_dtypes.float8_e4m3
    wqT = np.ascontiguousarray(wq.reshape(C, C).T.astype(FP8NP))  # [C_in, NH*CH]
    wkT = np.ascontiguousarray(wk.reshape(C, C).T.astype(FP8NP))
    wvT = np.ascontiguousarray(wv.reshape(C, C).T.astype(FP8NP))
    pwT = np.ascontiguousarray((proj_w * ISQ2).T.astype(ml_dtypes.bfloat16))
    pb = proj_b * ISQ2

    ind16 = np.zeros((C, G), np.float32)
    ind16[np.arange(C), np.arange(C) // GS] = 1.0 / GS
    # indT rows hold the HALF-LOCAL group index (group % 16): the GroupNorm
    # broadcast runs per chunk-pair half with group stats in partitions
    # 0:16, so every chunk's indicator lives in rows 0:16.
    indT = np.zeros((G, C), np.float32)
    indT[(np.arange(C) // GS) % (G // 2), np.arange(C)] = 1.0

    return dict(
        wqT=wqT, wkT=wkT, wvT=wvT, pwT=pwT,
        qb=qb, kb=kb, vb=vb, pb=pb,
        ind16=ind16, indT=indT,
    )


def kernel(**inputs):
    x = np.asarray(inputs["x"], np.float32)
    gn_w = np.asarray(inputs["gn_w"], np.float32)
    gn_b = np.asarray(inputs["gn_b"], np.float32)
    qkv_b = np.asarray(inputs["qkv_b"], np.float32)
    proj_b = np.asarray(inputs["proj_b"], np.float32)

    prep = _host_prep(inputs["qkv_w"], qkv_b, inputs["proj_w"], proj_b)
    qkv_bias_nz = bool(np.any(qkv_b != 0))
    proj_bias_nz = bool(np.any(proj_b != 0))

    key = (qkv_bias_nz, proj_bias_nz)
    if key not in _GRAPH_CACHE:
        _GRAPH_CACHE[key] = _build_graph(qkv_bias_nz, proj_bias_nz)
    nc = _GRAPH_CACHE[key]

    shared = dict(
        wqT=prep["wqT"], wkT=prep["wkT"], wvT=prep["wvT"], pwT=prep["pwT"],
        gnw=gn_w, gnb=gn_b, ind16=prep["ind16"], indT=prep["indT"],
    )
    if qkv_bias_nz:
        shared.update(qb=prep["qb"], kb=prep["kb"], vb=prep["vb"])
    if proj_bias_nz:
        shared.update(pb=prep["pb"])

    in_maps = [
        {**shared, "x": np.ascontiguousarray(x[i].reshape(C, T))}
        for i in range(NCORES)
    ]
    res = run_bass_kernel_spmd(nc, in_maps, core_ids=list(range(NCORES)))
    out = np.stack(
        [res.results[i]["out"].reshape(C, 32, 32) for i in range(NCORES)]
    )
    kernel._last_results = res
    return out
